# revision 42
# baseline (speedup 1.0000x reference)
"""Distributed Trainium2 Bass kernel for multi-head attention w/ RoPE.

Reference op (B=4, S=2048, D=1024, H=16, HD=64, fp32):
    q/k/v = hidden @ W{q,k,v}.T + b   (per-head reshape)
    q, k  = rope(q), rope(k)
    out   = softmax(q k^T / sqrt(HD)) v  @ Wo.T

Sharding v2: 8 cores = 4 batches x 2 head-groups (8 heads each). Every
core projects Q/K/V only for its own 512 features over the full 2048
tokens (no duplicated work anywhere -- PE row count is at the
theoretical floor of 786432 rows/core), runs attention for its 8 heads,
and o-projects its feature slice against the matching Wo rows. The two
half-outputs per batch are summed on the host (pure unshard add).

Single fused pipeline, fully transposed layout (features on partitions):
V projects first (natural layout, ones column appended so the softmax
denominator falls out of the attn@V matmul); then per head-pair: Q/K^T
projection chunks -> RoPE (DVE muls + a batched DMA partition band-swap
+ adds). K lands in TWO zero-padded stationary tiles (even head in rows
0:64 of ke, odd head in rows 64:128 of ko, other half zero via
parity-masked cos tables) so every scores matmul is a full 128-row
(128,128) PE tile against the full 128-row qtile moving operand --
avoiding the ~150ns PE reconfigure penalty that 64-row stationaries pay
on every row-size switch. Scores -> wide [128,1024] exp on ACT (scale
1/8 folded in, ACT does nothing else) -> attn@V interleaved one k-chunk
pair behind so PE fills ACT's exp latency. Normalization is
evicted-early (DVE copy frees PSUM), flushed one q-block late: exact
reciprocal runs 64-wide, gpsimd hops/broadcasts it, odd heads hop into
the o-proj operand via DMA. The o-projection for the last head-pair is
pipelined per q-block behind the final attention sweeps; output is
written bf16 and upcast host-side. All matmuls bf16, fp32 accumulation.
Nonzero biases ride an augmented K=1 contraction row (skipped when the
caller's biases are all zero).
"""

import sys

import numpy as np

try:  # concourse ships in the container; fall back to the staged repo
    import concourse.bass  # noqa: F401
except Exception:  # pragma: no cover
    sys.path.insert(0, "/opt/trn_rl_repo")

import ml_dtypes

B, S, D, H = 4, 2048, 1024, 16
HD = D // H                      # 64
P = 128
NCORES = 8
SK = S                           # 2048 tokens per core (q and k)
DO = 512                         # per-core head-group width (8 heads)
HC = 8                           # heads per core
ND = D // P                      # 8 feature contraction chunks
NPI = DO // P                    # 4 head-pair chunks
NT = SK // P                     # 16 key/token chunks
QF = 512                         # matmul moving width
NQF = SK // QF                   # 4 query blocks
ROPE_BASE = 10000.0
BF16 = ml_dtypes.bfloat16

TRACE = False                    # test harness flips this
TRACE_KW = {}
LAST = {}                        # exec_time_ns / trace path for test harness

_cache = {}


def _build_nc(with_bias):
    import concourse.bass as bass
    import concourse.mybir as mybir
    import concourse.tile as tile
    from concourse import bacc
    from contextlib import ExitStack

    f32 = mybir.dt.float32
    bf16 = mybir.dt.bfloat16
    AF = mybir.ActivationFunctionType
    PSUM = bass.MemorySpace.PSUM

    nc = bacc.Bacc(None)
    xT = nc.declare_dram_parameter("xT", [D + 1, SK], bf16, False)
    # weights are host-prearranged so every DMA is one fat contiguous
    # descriptor per partition (the natural W.T slices would shatter
    # into 1024 x 256B descriptors and clog all 16 DMA queues)
    wqT = nc.declare_dram_parameter("wqT", [P, NPI, ND, P], bf16, False)
    wkT = nc.declare_dram_parameter("wkT", [P, NPI, ND, P], bf16, False)
    wvT = nc.declare_dram_parameter("wvT", [P, ND, DO], bf16, False)
    woT = nc.declare_dram_parameter("woT", [P, NPI, D], bf16, False)
    wqb = nc.declare_dram_parameter("wqb", [1, DO], bf16, False)
    wkb = nc.declare_dram_parameter("wkb", [1, DO], bf16, False)
    wvb = nc.declare_dram_parameter("wvb", [1, DO], bf16, False)
    cosk = nc.declare_dram_parameter("cosk", [P, SK], bf16, False)
    sink = nc.declare_dram_parameter("sink", [P, SK], bf16, False)
    out = nc.declare_dram_parameter("out", [SK, D], bf16, True)

    with tile.TileContext(nc) as tc, ExitStack() as st:
        sb = st.enter_context(tc.tile_pool(name="sb", bufs=1))
        qk = st.enter_context(tc.tile_pool(name="qk", bufs=2))
        wp = st.enter_context(tc.tile_pool(name="wp", bufs=2))
        tp = st.enter_context(tc.tile_pool(name="tp", bufs=2))
        etp = st.enter_context(tc.tile_pool(name="et", bufs=6))
        npool = st.enter_context(tc.tile_pool(name="nrm", bufs=3))
        outp = st.enter_context(tc.tile_pool(name="ou", bufs=3))
        psp = st.enter_context(tc.tile_pool(name="ps", bufs=2, space=PSUM))

        vst = [sb.tile([P, HC, HD + 1], bf16, tag=f"v{t}", name=f"v{t}")
               for t in range(NT)]
        ones64 = sb.tile([1, HD], f32, tag="one64", name="one64")
        nc.vector.memset(ones64[:], 1.0)
        at = [sb.tile([P, SK], bf16, tag=f"at{i}", name=f"at{i}")
              for i in range(NPI)]

        # ---- loads (issue order = need order: pi0 weight slices first
        # so the projection chains can chase the x^T chunk DMAs) --------
        def load_wslice(wdram, wbdram, pi, wtag):
            ws = wp.tile([P, ND, P], bf16, tag=wtag, name=wtag)
            nc.sync.dma_start(out=ws[:], in_=wdram[:, pi, :, :])
            wb = None
            if with_bias:
                wb = wp.tile([1, P], bf16, tag=wtag + "b", name=wtag + "b")
                nc.sync.dma_start(out=wb[:], in_=wbdram[:, pi * P:(pi + 1) * P])
            return ws, wb

        wnext = (load_wslice(wqT, wqb, 0, "wq"), load_wslice(wkT, wkb, 0, "wk"))
        xs = [sb.tile([P, SK], bf16, tag=f"x{d}", name=f"x{d}")
              for d in range(ND)]
        for d_ in range(ND):
            nc.sync.dma_start(out=xs[d_][:], in_=xT[d_ * P:(d_ + 1) * P, :])
        if with_bias:
            xone = sb.tile([1, SK], bf16, tag="xone", name="xone")
            nc.sync.dma_start(out=xone[:], in_=xT[D:D + 1, :])
        # everything else queues on sync BEHIND x^T: the 16 DMA rings
        # round-robin all outstanding descriptors, so issuing these from
        # another queue would steal bandwidth from the critical-path x^T
        # chunks the first projection chains are chasing
        ck = sb.tile([P, SK], bf16, tag="ck", name="ck")
        sk_ = sb.tile([P, SK], bf16, tag="sk", name="sk")
        nc.sync.dma_start(out=ck[:], in_=cosk[:, :])
        nc.sync.dma_start(out=sk_[:], in_=sink[:, :])
        # zero the off-parity halves of the ke/ko pool buffers once (the
        # rope writes never touch them), instead of shipping masked cos
        # tables -- saves 1MB of critical-path DMA
        for _ in range(2):
            tke = qk.tile([P, SK], bf16, tag="ke", name="kez")
            nc.vector.memset(tke[HD:P, :], 0.0)
            tko = qk.tile([P, SK], bf16, tag="ko", name="koz")
            nc.vector.memset(tko[0:HD, :], 0.0)
        wv = wp.tile([P, ND, DO], bf16, tag="wv", name="wv", bufs=1)
        nc.sync.dma_start(out=wv[:], in_=wvT[:])
        if with_bias:
            wvbt = wp.tile([1, DO], bf16, tag="wvb", name="wvb", bufs=1)
            nc.sync.dma_start(out=wvbt[:], in_=wvb[:])
        wo = wp.tile([P, NPI, D], bf16, tag="wo", name="wo", bufs=1)
        nc.sync.dma_start(out=wo[:], in_=woT[:])

        def qk_proj(wsb, dst, dsto=None, dmaq=None):
            """dst = rope(W[pi-slice] @ x^T + b). Q path (dsto None):
            full-width writes into dst. K path: even head -> dst rows
            0:64 (rows 64:128 stay zero via the masked cos table), odd
            head -> dsto rows 64:128 -- zero-padded 128-row stationaries
            for the scores matmuls."""
            ws, wb = wsb
            t2 = tp.tile([P, SK], bf16, tag="t2", name="t2")
            t2s = tp.tile([P, SK], bf16, tag="t2s", name="t2s")
            for c in range(SK // QF):
                ps = psp.tile([P, QF], f32, tag="pp", name="pp")
                for d_ in range(ND):
                    nc.tensor.matmul(
                        ps[:], ws[:, d_, :], xs[d_][:, c * QF:(c + 1) * QF],
                        start=(d_ == 0), stop=(not with_bias and d_ == ND - 1))
                if with_bias:
                    nc.tensor.matmul(
                        ps[:], wb[:], xone[:, c * QF:(c + 1) * QF],
                        start=False, stop=True)
                cs = slice(c * QF, (c + 1) * QF)
                if dsto is None:
                    nc.vector.tensor_mul(dst[:, cs], ps[:], ck[:, cs])
                else:
                    nc.vector.tensor_mul(
                        dst[0:HD, cs], ps[0:HD, :], ck[0:HD, cs])
                    nc.vector.tensor_mul(
                        dsto[HD:P, cs], ps[HD:P, :], ck[HD:P, cs])
                nc.vector.tensor_mul(t2[:, cs], ps[:], sk_[:, cs])
                if c % 2 == 1:
                    # band swap d<->d+32 (pi0 rides the scalar DMA queue
                    # while sync drains the x^T loads; later pi use sync
                    # so swaps never queue behind exp issues) + add,
                    # batched over the finished 1024-wide half
                    hs_ = slice((c - 1) * QF, (c + 1) * QF)
                    for b0 in (0, 64):
                        dmaq.dma_start(
                            out=t2s[b0:b0 + 32, hs_], in_=t2[b0 + 32:b0 + 64, hs_])
                        dmaq.dma_start(
                            out=t2s[b0 + 32:b0 + 64, hs_], in_=t2[b0:b0 + 32, hs_])
                    if dsto is None:
                        nc.vector.tensor_add(
                            dst[:, hs_], dst[:, hs_], t2s[:, hs_])
                    else:
                        nc.vector.tensor_add(
                            dst[0:HD, hs_], dst[0:HD, hs_], t2s[0:HD, hs_])
                        nc.vector.tensor_add(
                            dsto[HD:P, hs_], dsto[HD:P, hs_], t2s[HD:P, hs_])

        def v_proj():
            # V projection (natural layout, x^T stationary)
            for t_ in range(NT):
                ps = psp.tile([P, DO], f32, tag="pp", name="pp")
                for d_ in range(ND):
                    nc.tensor.matmul(
                        ps[:], xs[d_][:, t_ * P:(t_ + 1) * P], wv[:, d_, :],
                        start=(d_ == 0), stop=(not with_bias and d_ == ND - 1))
                if with_bias:
                    nc.tensor.matmul(
                        ps[:], xone[:, t_ * P:(t_ + 1) * P], wvbt[:],
                        start=False, stop=True)
                nc.vector.tensor_copy(
                    vst[t_][:, :, 0:HD],
                    ps[:].rearrange("p (h d) -> p h d", d=HD))
                nc.vector.memset(vst[t_][:, :, HD:HD + 1], 1.0)

        # ---- fused per-head-pair projection + attention ----------------
        pend = []

        def flush_one():
            # normalize in SBUF: exact reciprocal spread 64-wide (~0.3us
            # not 3.3us single-lane), DMA-hop to p0, partition-
            # broadcast, multiply into the o-proj operand. The even
            # head's hops ride the sync queue so the two parity chains
            # drain in parallel (gpsimd serializes its own hops)
            pi, qqs, osb_e, osb_o, last = pend.pop(0)
            for par, osb, dq in ((0, osb_e, nc.sync), (1, osb_o, nc.gpsimd)):
                smr = npool.tile([HD, 8], f32, tag="smr", name="smr")
                dq.dma_start(out=smr[:], in_=osb[HD:HD + 1, :])
                rcs = npool.tile([HD, 8], f32, tag="rcs", name="rcs")
                nc.vector.reciprocal(rcs[:], smr[:])
                rc = npool.tile([1, QF], f32, tag="rc", name="rc")
                dq.dma_start(out=rc[:], in_=rcs[:])
                if last:
                    # the final flush is latency-exposed: broadcast on
                    # the (idle) PE via a ones stationary instead of the
                    # ~1.1us gpsimd PartitionBroadcast
                    bcp = psp.tile([HD + 1, QF], f32, tag="o", name="bcp")
                    nc.tensor.matmul(bcp[0:HD, :], ones64[:], rc[:],
                                     start=True, stop=True)
                    bc = bcp[0:HD, :]
                else:
                    bct = npool.tile([HD, QF], f32, tag="bc", name="bc")
                    nc.gpsimd.partition_broadcast(bct[:], rc[:])
                    bc = bct[:]
                if par == 0:
                    nc.vector.tensor_mul(
                        at[pi][0:HD, qqs], osb[0:HD, :], bc)
                else:
                    # odd heads land at partition base 64; a pure-SBUF
                    # base-shifted DVE write corrupts, so write at base
                    # 0 and DMA-hop into place (sync queue: it gates the
                    # last head-pair's o-projection)
                    atm = npool.tile([HD, QF], bf16, tag="atm", name="atm")
                    nc.vector.tensor_mul(atm[:], osb[0:HD, :], bc)
                    nc.sync.dma_start(out=at[pi][HD:P, qqs], in_=atm[:])

        def _oproj_close(view, qa, oh, act=False):
            # the final q block evicts via ACT (its exp work is done and
            # DVE still has flush multiplies in flight); earlier blocks
            # stay on DVE since ACT is still running the next attention
            # block's exps
            ob = outp.tile([P, QF], bf16, tag="ob", name="ob")
            if act:
                nc.scalar.activation(ob[:], view, AF.Copy)
            else:
                nc.vector.tensor_copy(ob[:], view)
            dq = nc.scalar if (qa + oh) % 2 == 0 else nc.sync
            dq.dma_start(
                out=out[qa * P:(qa + 1) * P, oh * QF:(oh + 1) * QF],
                in_=ob[:])

        def oproj(qh):
            # o-projection for one 512-wide q block; consumes the
            # transposed at[] tiles directly, writes bf16
            for qc in range(QF // P):
                qa = qh * (QF // P) + qc
                for oh in range(2):
                    ps = psp.tile([P, QF], f32, tag="pp", name="pp")
                    for f in range(NPI):
                        nc.tensor.matmul(
                            ps[:], at[f][:, qa * P:(qa + 1) * P],
                            wo[:, f, oh * QF:(oh + 1) * QF],
                            start=(f == 0), stop=(f == NPI - 1))
                    _oproj_close(ps[:], qa, oh)

        def oproj_last():
            # final q block: the f<3 partials of the first chains are
            # emitted open (no stop) so the PE chews them while the last
            # flush chain drains; only the f=3 matmuls wait on at[3].
            # Free "s"-tag banks host 4 of the early chains
            slots = [(qh_ * 0 + (NQF - 1) * (QF // P) + qc, oh)
                     for qc in range(QF // P) for oh in range(2)
                     for qh_ in (0,)]
            views = []
            for i, (qa, oh) in enumerate(slots[:6]):
                if i < 2:
                    ps = psp.tile([P, QF], f32, tag="pp", name="pp")
                    views.append(ps[:])
                else:
                    if i % 2 == 0:
                        stile = psp.tile([P, 2 * QF], f32, tag="s", name="s")
                    views.append(stile[:, (i % 2) * QF:(i % 2 + 1) * QF])
                for f in range(NPI - 1):
                    nc.tensor.matmul(
                        views[i], at[f][:, qa * P:(qa + 1) * P],
                        wo[:, f, oh * QF:(oh + 1) * QF],
                        start=(f == 0), stop=False, skip_group_check=True)
            for i, (qa, oh) in enumerate(slots[:6]):
                nc.tensor.matmul(
                    views[i], at[NPI - 1][:, qa * P:(qa + 1) * P],
                    wo[:, NPI - 1, oh * QF:(oh + 1) * QF],
                    start=False, stop=True, skip_group_check=True)
                _oproj_close(views[i], qa, oh)
            for qa, oh in slots[6:]:
                ps = psp.tile([P, QF], f32, tag="pp", name="pp")
                for f in range(NPI):
                    nc.tensor.matmul(
                        ps[:], at[f][:, qa * P:(qa + 1) * P],
                        wo[:, f, oh * QF:(oh + 1) * QF],
                        start=(f == 0), stop=(f == NPI - 1))
                _oproj_close(ps[:], qa, oh)

        def do_qk(wsb_pair, dmaq):
            qtile = qk.tile([P, SK], bf16, tag="qt", name="qt")
            qk_proj(wsb_pair[0], qtile, dmaq=dmaq)
            ke = qk.tile([P, SK], bf16, tag="ke", name="ke")
            ko = qk.tile([P, SK], bf16, tag="ko", name="ko")
            qk_proj(wsb_pair[1], ke, ko, dmaq=dmaq)
            return qtile, ke, ko

        # software pipeline: the next head-pair's Q/K projection (PE
        # chains + DVE rope + swap DMAs) is emitted before the CURRENT
        # pair's last attention block, so its rope pipeline drains while
        # the PE is still busy -- no dead time at head-pair boundaries
        cur = do_qk(wnext, nc.scalar)
        v_proj()
        nxt = None
        wnext = (load_wslice(wqT, wqb, 1, "wq"), load_wslice(wkT, wkb, 1, "wk"))
        for pi in range(NPI):
            qtile, ke, ko = cur

            for qh in range(NQF):
                if qh == NQF - 1 and pi + 1 < NPI:
                    nxt = do_qk(wnext, nc.sync)
                    if pi + 2 < NPI:
                        wnext = (load_wslice(wqT, wqb, pi + 2, "wq"),
                                 load_wslice(wkT, wkb, pi + 2, "wk"))
                qs = slice(qh * QF, (qh + 1) * QF)
                ope = psp.tile([HD + 1, QF], f32, tag="o", name="o")
                opo = psp.tile([HD + 1, QF], f32, tag="o", name="o")
                prev = None
                for kcp in range(NT // 2):
                    spe = psp.tile([P, 2 * QF], f32, tag="s", name="s")
                    spo = psp.tile([P, 2 * QF], f32, tag="s", name="s")
                    for j in range(2):
                        ks_ = slice((2 * kcp + j) * P, (2 * kcp + j + 1) * P)
                        js = slice(j * QF, (j + 1) * QF)
                        nc.tensor.matmul(
                            spe[:, js], ke[:, ks_], qtile[:, qs],
                            start=True, stop=True)
                        nc.tensor.matmul(
                            spo[:, js], ko[:, ks_], qtile[:, qs],
                            start=True, stop=True)
                    ee = etp.tile([P, 2 * QF], bf16, tag="e", name="e")
                    eo = etp.tile([P, 2 * QF], bf16, tag="e", name="e")
                    for j in range(2):
                        js = slice(j * QF, (j + 1) * QF)
                        nc.scalar.activation(
                            ee[:, js], spe[:, js], AF.Exp, scale=0.125)
                        nc.scalar.activation(
                            eo[:, js], spo[:, js], AF.Exp, scale=0.125)
                    # attn@V for the previous k-chunk pair overlaps this
                    # pair's exp latency on the PE
                    if prev is not None:
                        pee, peo, pk = prev
                        for j in range(2):
                            kc = 2 * pk + j
                            js = slice(j * QF, (j + 1) * QF)
                            nc.tensor.matmul(
                                ope[:], vst[kc][:, 2 * pi, :], pee[:, js],
                                start=(kc == 0), stop=False)
                            nc.tensor.matmul(
                                opo[:], vst[kc][:, 2 * pi + 1, :], peo[:, js],
                                start=(kc == 0), stop=False)
                    prev = (ee, eo, kcp)
                pee, peo, pk = prev
                for j in range(2):
                    kc = 2 * pk + j
                    js = slice(j * QF, (j + 1) * QF)
                    nc.tensor.matmul(
                        ope[:], vst[kc][:, 2 * pi, :], pee[:, js],
                        start=False, stop=(kc == NT - 1))
                    nc.tensor.matmul(
                        opo[:], vst[kc][:, 2 * pi + 1, :], peo[:, js],
                        start=False, stop=(kc == NT - 1))

                # evict PSUM immediately (quick DVE copies free the "o"
                # slots), then flush the reciprocal chain right away --
                # its gpsimd/DVE latency hides under the next attention
                # block. The last head-pair's o-projection trails one
                # q-block so flush(qh-1) has a full block to complete.
                osb_e = npool.tile([HD + 1, QF], f32, tag="osb", name="osb",
                                   bufs=6)
                nc.vector.tensor_copy(osb_e[:], ope[:])
                osb_o = npool.tile([HD + 1, QF], f32, tag="osb", name="osb",
                                   bufs=6)
                nc.vector.tensor_copy(osb_o[:], opo[:])
                pend.append((pi, qs, osb_e, osb_o,
                             pi == NPI - 1 and qh == NQF - 1))
                flush_one()
                if pi == NPI - 1 and qh > 0:
                    oproj(qh - 1)
            cur, nxt = nxt, None

        oproj_last()
    nc.compile()
    return nc


def _rope_tables(pos):
    """pos [n] -> cos/sin tables [128, n] bf16 (sign-folded sin)."""
    inv = ROPE_BASE ** (-np.arange(0, HD, 2, dtype=np.float64) / HD)
    fr = np.outer(pos.astype(np.float64), inv)          # [n, 32]
    c, s = np.cos(fr), np.sin(fr)
    cos64 = np.concatenate([c, c], axis=1).T            # [64, n]
    sinA = np.concatenate([s, -s], axis=1).T            # [64, n]
    ck = np.concatenate([cos64, cos64], axis=0).astype(BF16)
    sk = np.concatenate([sinA, sinA], axis=0).astype(BF16)
    return ck, sk


def _prep_wqk(w, g):
    """[D, D] weight -> bf16 [P, NPI, ND, P]: W.T columns for head group
    g, prearranged so each per-pi stationary DMA is one contiguous 2KB
    descriptor per partition."""
    wt = np.asarray(w, dtype=np.float32).T[:, g * DO:(g + 1) * DO]
    arr = wt.reshape(ND, P, NPI, P).transpose(1, 2, 0, 3)
    return np.ascontiguousarray(arr).astype(BF16)


def _prep_wv(w, g):
    """[D, D] weight -> bf16 [P, ND, DO] (prearranged W.T columns)."""
    wt = np.asarray(w, dtype=np.float32).T[:, g * DO:(g + 1) * DO]
    arr = wt.reshape(ND, P, DO).transpose(1, 0, 2)
    return np.ascontiguousarray(arr).astype(BF16)


def _prep_wo(w, g):
    """[D, D] weight -> bf16 [P, NPI, D]: W.T rows for head group g."""
    wt = np.asarray(w, dtype=np.float32).T[g * DO:(g + 1) * DO, :]
    arr = wt.reshape(NPI, P, D).transpose(1, 0, 2)
    return np.ascontiguousarray(arr).astype(BF16)


def _prep_b(b, g):
    return np.ascontiguousarray(
        np.asarray(b, dtype=np.float32)[None, g * DO:(g + 1) * DO]).astype(BF16)


def kernel(hidden_states, position_ids, Wq, bq, Wk, bk, Wv, bv, Wo):
    from concourse import bass_utils

    with_bias = bool(
        np.any(np.asarray(bq)) or np.any(np.asarray(bk)) or np.any(np.asarray(bv)))
    key = ("nc", with_bias)
    if key not in _cache:
        _cache[key] = _build_nc(with_bias)
    nc = _cache[key]

    hs = np.asarray(hidden_states, dtype=np.float32)
    pos = np.asarray(position_ids)
    wq = [_prep_wqk(Wq, g) for g in range(2)]
    wk = [_prep_wqk(Wk, g) for g in range(2)]
    wv = [_prep_wv(Wv, g) for g in range(2)]
    wo = [_prep_wo(Wo, g) for g in range(2)]
    bqs = [_prep_b(bq, g) for g in range(2)]
    bks = [_prep_b(bk, g) for g in range(2)]
    bvs = [_prep_b(bv, g) for g in range(2)]

    xts, tabs = [], []
    for b in range(B):
        xT = np.empty((D + 1, SK), dtype=np.float32)
        xT[:D] = hs[b].T
        xT[D] = 1.0
        xts.append(np.ascontiguousarray(xT).astype(BF16))
        tabs.append(_rope_tables(np.asarray(pos[b])))

    in_maps = []
    for core in range(NCORES):
        b, g = core // 2, core % 2
        ck, sk = tabs[b]
        in_maps.append({
            "xT": xts[b], "wqT": wq[g], "wkT": wk[g], "wvT": wv[g],
            "woT": wo[g], "wqb": bqs[g], "wkb": bks[g], "wvb": bvs[g],
            "cosk": ck, "sink": sk,
        })

    res = bass_utils.run_bass_kernel_spmd(
        nc, in_maps, core_ids=list(range(NCORES)), trace=TRACE, **TRACE_KW)
    LAST["exec_time_ns"] = res.exec_time_ns
    LAST["mean_exec_time_ns"] = res.mean_exec_time_ns
    LAST["trace"] = res.instructions_and_trace
    LAST["profile_json"] = res.profile_json

    outp_full = np.empty((B, S, D), dtype=np.float32)
    for b in range(B):
        outp_full[b] = (
            np.asarray(res.results[2 * b]["out"], dtype=np.float32)
            + np.asarray(res.results[2 * b + 1]["out"], dtype=np.float32))
    return outp_full


# revision 43
# speedup vs baseline: 1.1545x; 1.1545x over previous
"""Distributed Trainium2 Bass kernel for multi-head attention w/ RoPE.

Reference op (B=4, S=2048, D=1024, H=16, HD=64, fp32):
    q/k/v = hidden @ W{q,k,v}.T + b   (per-head reshape)
    q, k  = rope(q), rope(k)
    out   = softmax(q k^T / sqrt(HD)) v  @ Wo.T

Sharding v2: 8 cores = 4 batches x 2 head-groups (8 heads each). Every
core projects Q/K/V only for its own 512 features over the full 2048
tokens (no duplicated work anywhere -- PE row count is at the
theoretical floor of 786432 rows/core), runs attention for its 8 heads,
and o-projects its feature slice against the matching Wo rows. The two
half-outputs per batch are summed on the host (pure unshard add).

Single fused pipeline, fully transposed layout (features on partitions):
V projects first (natural layout, ones column appended so the softmax
denominator falls out of the attn@V matmul); then per head-pair: Q/K^T
projection chunks -> RoPE (DVE muls + a batched DMA partition band-swap
+ adds). K lands in TWO zero-padded stationary tiles (even head in rows
0:64 of ke, odd head in rows 64:128 of ko, other half zero via
parity-masked cos tables) so every scores matmul is a full 128-row
(128,128) PE tile against the full 128-row qtile moving operand --
avoiding the ~150ns PE reconfigure penalty that 64-row stationaries pay
on every row-size switch. Scores -> wide [128,1024] exp on ACT (scale
1/8 folded in, ACT does nothing else) -> attn@V interleaved one k-chunk
pair behind so PE fills ACT's exp latency. Normalization is
evicted-early (DVE copy frees PSUM), flushed one q-block late: exact
reciprocal runs 64-wide, gpsimd hops/broadcasts it, odd heads hop into
the o-proj operand via DMA. The o-projection for the last head-pair is
pipelined per q-block behind the final attention sweeps; output is
written bf16 and upcast host-side. All matmuls bf16, fp32 accumulation.
Nonzero biases ride an augmented K=1 contraction row (skipped when the
caller's biases are all zero).
"""

import sys

import numpy as np

try:  # concourse ships in the container; fall back to the staged repo
    import concourse.bass  # noqa: F401
except Exception:  # pragma: no cover
    sys.path.insert(0, "/opt/trn_rl_repo")

import ml_dtypes

B, S, D, H = 4, 2048, 1024, 16
HD = D // H                      # 64
P = 128
NCORES = 8
SK = S                           # 2048 tokens per core (q and k)
DO = 512                         # per-core head-group width (8 heads)
HC = 8                           # heads per core
ND = D // P                      # 8 feature contraction chunks
NPI = DO // P                    # 4 head-pair chunks
NT = SK // P                     # 16 key/token chunks
QF = 512                         # matmul moving width
NQF = SK // QF                   # 4 query blocks
ROPE_BASE = 10000.0
BF16 = ml_dtypes.bfloat16

TRACE = False                    # test harness flips this
TRACE_KW = {}
LAST = {}                        # exec_time_ns / trace path for test harness

_cache = {}


def _build_nc(with_bias):
    import concourse.bass as bass
    import concourse.mybir as mybir
    import concourse.tile as tile
    from concourse import bacc
    from contextlib import ExitStack

    f32 = mybir.dt.float32
    bf16 = mybir.dt.bfloat16
    AF = mybir.ActivationFunctionType
    PSUM = bass.MemorySpace.PSUM

    nc = bacc.Bacc(None)
    xT = nc.declare_dram_parameter("xT", [D + 1, SK], bf16, False)
    # weights are host-prearranged so every DMA is one fat contiguous
    # descriptor per partition (the natural W.T slices would shatter
    # into 1024 x 256B descriptors and clog all 16 DMA queues)
    wqT = nc.declare_dram_parameter("wqT", [P, NPI, ND, P], bf16, False)
    wkT = nc.declare_dram_parameter("wkT", [P, NPI, ND, P], bf16, False)
    wvT = nc.declare_dram_parameter("wvT", [P, ND, DO], bf16, False)
    woT = nc.declare_dram_parameter("woT", [P, NPI, D], bf16, False)
    wqb = nc.declare_dram_parameter("wqb", [1, DO], bf16, False)
    wkb = nc.declare_dram_parameter("wkb", [1, DO], bf16, False)
    wvb = nc.declare_dram_parameter("wvb", [1, DO], bf16, False)
    cosk = nc.declare_dram_parameter("cosk", [P, SK], bf16, False)
    sink = nc.declare_dram_parameter("sink", [P, SK], bf16, False)
    out = nc.declare_dram_parameter("out", [SK, D], bf16, True)

    with tile.TileContext(nc) as tc, ExitStack() as st:
        sb = st.enter_context(tc.tile_pool(name="sb", bufs=1))
        qk = st.enter_context(tc.tile_pool(name="qk", bufs=2))
        wp = st.enter_context(tc.tile_pool(name="wp", bufs=2))
        tp = st.enter_context(tc.tile_pool(name="tp", bufs=2))
        etp = st.enter_context(tc.tile_pool(name="et", bufs=6))
        npool = st.enter_context(tc.tile_pool(name="nrm", bufs=3))
        outp = st.enter_context(tc.tile_pool(name="ou", bufs=3))
        psp = st.enter_context(tc.tile_pool(name="ps", bufs=2, space=PSUM))

        vst = [sb.tile([P, HC, HD + 1], bf16, tag=f"v{t}", name=f"v{t}")
               for t in range(NT)]
        ones64 = sb.tile([1, HD], f32, tag="one64", name="one64")
        nc.vector.memset(ones64[:], 1.0)
        at = [sb.tile([P, SK], bf16, tag=f"at{i}", name=f"at{i}")
              for i in range(NPI)]

        # ---- loads (issue order = need order: pi0 weight slices first
        # so the projection chains can chase the x^T chunk DMAs) --------
        def load_wslice(wdram, wbdram, pi, wtag):
            ws = wp.tile([P, ND, P], bf16, tag=wtag, name=wtag)
            nc.sync.dma_start(out=ws[:], in_=wdram[:, pi, :, :])
            wb = None
            if with_bias:
                wb = wp.tile([1, P], bf16, tag=wtag + "b", name=wtag + "b")
                nc.sync.dma_start(out=wb[:], in_=wbdram[:, pi * P:(pi + 1) * P])
            return ws, wb

        wnext = (load_wslice(wqT, wqb, 0, "wq"), load_wslice(wkT, wkb, 0, "wk"))
        xs = [sb.tile([P, SK], bf16, tag=f"x{d}", name=f"x{d}")
              for d in range(ND)]
        for d_ in range(ND):
            nc.sync.dma_start(out=xs[d_][:], in_=xT[d_ * P:(d_ + 1) * P, :])
        if with_bias:
            xone = sb.tile([1, SK], bf16, tag="xone", name="xone")
            nc.sync.dma_start(out=xone[:], in_=xT[D:D + 1, :])
        # everything else queues on sync BEHIND x^T: the 16 DMA rings
        # round-robin all outstanding descriptors, so issuing these from
        # another queue would steal bandwidth from the critical-path x^T
        # chunks the first projection chains are chasing
        ck = sb.tile([P, SK], bf16, tag="ck", name="ck")
        sk_ = sb.tile([P, SK], bf16, tag="sk", name="sk")
        nc.sync.dma_start(out=ck[:], in_=cosk[:, :])
        nc.sync.dma_start(out=sk_[:], in_=sink[:, :])
        # zero the off-parity halves of the ke/ko pool buffers once (the
        # rope writes never touch them), instead of shipping masked cos
        # tables -- saves 1MB of critical-path DMA
        for _ in range(2):
            tke = qk.tile([P, SK], bf16, tag="ke", name="kez")
            nc.vector.memset(tke[HD:P, :], 0.0)
            tko = qk.tile([P, SK], bf16, tag="ko", name="koz")
            nc.vector.memset(tko[0:HD, :], 0.0)
        wv = wp.tile([P, ND, DO], bf16, tag="wv", name="wv", bufs=1)
        nc.sync.dma_start(out=wv[:], in_=wvT[:])
        if with_bias:
            wvbt = wp.tile([1, DO], bf16, tag="wvb", name="wvb", bufs=1)
            nc.sync.dma_start(out=wvbt[:], in_=wvb[:])
        wo = wp.tile([P, NPI, D], bf16, tag="wo", name="wo", bufs=1)
        nc.sync.dma_start(out=wo[:], in_=woT[:])

        def qk_proj(wsb, dst, dsto=None, dmaq=None):
            """dst = rope(W[pi-slice] @ x^T + b). Q path (dsto None):
            full-width writes into dst. K path: even head -> dst rows
            0:64 (rows 64:128 stay zero via the masked cos table), odd
            head -> dsto rows 64:128 -- zero-padded 128-row stationaries
            for the scores matmuls."""
            ws, wb = wsb
            t2 = tp.tile([P, SK], bf16, tag="t2", name="t2")
            t2s = tp.tile([P, SK], bf16, tag="t2s", name="t2s")
            for c in range(SK // QF):
                ps = psp.tile([P, QF], f32, tag="pp", name="pp")
                for d_ in range(ND):
                    nc.tensor.matmul(
                        ps[:], ws[:, d_, :], xs[d_][:, c * QF:(c + 1) * QF],
                        start=(d_ == 0), stop=(not with_bias and d_ == ND - 1))
                if with_bias:
                    nc.tensor.matmul(
                        ps[:], wb[:], xone[:, c * QF:(c + 1) * QF],
                        start=False, stop=True)
                cs = slice(c * QF, (c + 1) * QF)
                if dsto is None:
                    nc.vector.tensor_mul(dst[:, cs], ps[:], ck[:, cs])
                else:
                    nc.vector.tensor_mul(
                        dst[0:HD, cs], ps[0:HD, :], ck[0:HD, cs])
                    nc.vector.tensor_mul(
                        dsto[HD:P, cs], ps[HD:P, :], ck[HD:P, cs])
                nc.vector.tensor_mul(t2[:, cs], ps[:], sk_[:, cs])
                if c % 2 == 1:
                    # band swap d<->d+32 (pi0 rides the scalar DMA queue
                    # while sync drains the x^T loads; later pi use sync
                    # so swaps never queue behind exp issues) + add,
                    # batched over the finished 1024-wide half
                    hs_ = slice((c - 1) * QF, (c + 1) * QF)
                    for b0 in (0, 64):
                        dmaq.dma_start(
                            out=t2s[b0:b0 + 32, hs_], in_=t2[b0 + 32:b0 + 64, hs_])
                        dmaq.dma_start(
                            out=t2s[b0 + 32:b0 + 64, hs_], in_=t2[b0:b0 + 32, hs_])
                    if dsto is None:
                        nc.vector.tensor_add(
                            dst[:, hs_], dst[:, hs_], t2s[:, hs_])
                    else:
                        nc.vector.tensor_add(
                            dst[0:HD, hs_], dst[0:HD, hs_], t2s[0:HD, hs_])
                        nc.vector.tensor_add(
                            dsto[HD:P, hs_], dsto[HD:P, hs_], t2s[HD:P, hs_])

        def v_proj():
            # V projection (natural layout, x^T stationary)
            for t_ in range(NT):
                ps = psp.tile([P, DO], f32, tag="pp", name="pp")
                for d_ in range(ND):
                    nc.tensor.matmul(
                        ps[:], xs[d_][:, t_ * P:(t_ + 1) * P], wv[:, d_, :],
                        start=(d_ == 0), stop=(not with_bias and d_ == ND - 1))
                if with_bias:
                    nc.tensor.matmul(
                        ps[:], xone[:, t_ * P:(t_ + 1) * P], wvbt[:],
                        start=False, stop=True)
                nc.vector.tensor_copy(
                    vst[t_][:, :, 0:HD],
                    ps[:].rearrange("p (h d) -> p h d", d=HD))
                nc.vector.memset(vst[t_][:, :, HD:HD + 1], 1.0)

        # ---- fused per-head-pair projection + attention ----------------
        pend = []

        def flush_one():
            # normalize in SBUF: exact reciprocal spread 64-wide (~0.3us
            # not 3.3us single-lane), DMA-hop to p0, partition-
            # broadcast, multiply into the o-proj operand. The even
            # head's hops ride the sync queue so the two parity chains
            # drain in parallel (gpsimd serializes its own hops)
            pi, qqs, osb_e, osb_o, last = pend.pop(0)
            for par, osb, dq in ((0, osb_e, nc.sync), (1, osb_o, nc.gpsimd)):
                smr = npool.tile([HD, 8], f32, tag="smr", name="smr")
                dq.dma_start(out=smr[:], in_=osb[HD:HD + 1, :])
                rcs = npool.tile([HD, 8], f32, tag="rcs", name="rcs")
                nc.vector.reciprocal(rcs[:], smr[:])
                rc = npool.tile([1, QF], f32, tag="rc", name="rc")
                dq.dma_start(out=rc[:], in_=rcs[:])
                if last:
                    # the final flush is latency-exposed: broadcast on
                    # the (idle) PE via a ones stationary instead of the
                    # ~1.1us gpsimd PartitionBroadcast
                    bcp = psp.tile([HD + 1, QF], f32, tag="o", name="bcp")
                    nc.tensor.matmul(bcp[0:HD, :], ones64[:], rc[:],
                                     start=True, stop=True)
                    bc = bcp[0:HD, :]
                else:
                    bct = npool.tile([HD, QF], f32, tag="bc", name="bc")
                    nc.gpsimd.partition_broadcast(bct[:], rc[:])
                    bc = bct[:]
                if par == 0:
                    nc.vector.tensor_mul(
                        at[pi][0:HD, qqs], osb[0:HD, :], bc)
                else:
                    # odd heads land at partition base 64; a pure-SBUF
                    # base-shifted DVE write corrupts, so write at base
                    # 0 and DMA-hop into place (sync queue: it gates the
                    # last head-pair's o-projection)
                    atm = npool.tile([HD, QF], bf16, tag="atm", name="atm")
                    nc.vector.tensor_mul(atm[:], osb[0:HD, :], bc)
                    nc.sync.dma_start(out=at[pi][HD:P, qqs], in_=atm[:])

        def _oproj_close(view, qa, oh, act=False):
            # the final q block evicts via ACT (its exp work is done and
            # DVE still has flush multiplies in flight); earlier blocks
            # stay on DVE since ACT is still running the next attention
            # block's exps
            ob = outp.tile([P, QF], bf16, tag="ob", name="ob")
            if act:
                nc.scalar.activation(ob[:], view, AF.Copy)
            else:
                nc.vector.tensor_copy(ob[:], view)
            dq = nc.scalar if (qa + oh) % 2 == 0 else nc.sync
            dq.dma_start(
                out=out[qa * P:(qa + 1) * P, oh * QF:(oh + 1) * QF],
                in_=ob[:])

        def oproj(qh):
            # o-projection for one 512-wide q block; consumes the
            # transposed at[] tiles directly, writes bf16
            for qc in range(QF // P):
                qa = qh * (QF // P) + qc
                for oh in range(2):
                    ps = psp.tile([P, QF], f32, tag="pp", name="pp")
                    for f in range(NPI):
                        nc.tensor.matmul(
                            ps[:], at[f][:, qa * P:(qa + 1) * P],
                            wo[:, f, oh * QF:(oh + 1) * QF],
                            start=(f == 0), stop=(f == NPI - 1))
                    _oproj_close(ps[:], qa, oh)

        def oproj_last():
            # final q block: the f<3 partials of the first chains are
            # emitted open (no stop) so the PE chews them while the last
            # flush chain drains; only the f=3 matmuls wait on at[3].
            # Free "s"-tag banks host 4 of the early chains
            slots = [(qh_ * 0 + (NQF - 1) * (QF // P) + qc, oh)
                     for qc in range(QF // P) for oh in range(2)
                     for qh_ in (0,)]
            views = []
            for i, (qa, oh) in enumerate(slots[:6]):
                if i < 2:
                    ps = psp.tile([P, QF], f32, tag="pp", name="pp")
                    views.append(ps[:])
                else:
                    if i % 2 == 0:
                        stile = psp.tile([P, 2 * QF], f32, tag="s", name="s")
                    views.append(stile[:, (i % 2) * QF:(i % 2 + 1) * QF])
                for f in range(NPI - 1):
                    nc.tensor.matmul(
                        views[i], at[f][:, qa * P:(qa + 1) * P],
                        wo[:, f, oh * QF:(oh + 1) * QF],
                        start=(f == 0), stop=False, skip_group_check=True)
            for i, (qa, oh) in enumerate(slots[:6]):
                nc.tensor.matmul(
                    views[i], at[NPI - 1][:, qa * P:(qa + 1) * P],
                    wo[:, NPI - 1, oh * QF:(oh + 1) * QF],
                    start=False, stop=True, skip_group_check=True)
                _oproj_close(views[i], qa, oh)
            for qa, oh in slots[6:]:
                ps = psp.tile([P, QF], f32, tag="pp", name="pp")
                for f in range(NPI):
                    nc.tensor.matmul(
                        ps[:], at[f][:, qa * P:(qa + 1) * P],
                        wo[:, f, oh * QF:(oh + 1) * QF],
                        start=(f == 0), stop=(f == NPI - 1))
                _oproj_close(ps[:], qa, oh)

        def do_qk(wsb_pair, dmaq):
            qtile = qk.tile([P, SK], bf16, tag="qt", name="qt")
            qk_proj(wsb_pair[0], qtile, dmaq=dmaq)
            ke = qk.tile([P, SK], bf16, tag="ke", name="ke")
            ko = qk.tile([P, SK], bf16, tag="ko", name="ko")
            qk_proj(wsb_pair[1], ke, ko, dmaq=dmaq)
            return qtile, ke, ko

        # software pipeline: the next head-pair's Q/K projection (PE
        # chains + DVE rope + swap DMAs) is emitted before the CURRENT
        # pair's last attention block, so its rope pipeline drains while
        # the PE is still busy -- no dead time at head-pair boundaries
        cur = do_qk(wnext, nc.scalar)
        v_proj()
        nxt = None
        wnext = (load_wslice(wqT, wqb, 1, "wq"), load_wslice(wkT, wkb, 1, "wk"))
        for pi in range(NPI):
            qtile, ke, ko = cur

            for qh in range(NQF):
                if qh == NQF - 1 and pi + 1 < NPI:
                    nxt = do_qk(wnext, nc.sync)
                    if pi + 2 < NPI:
                        wnext = (load_wslice(wqT, wqb, pi + 2, "wq"),
                                 load_wslice(wkT, wkb, pi + 2, "wk"))
                qs = slice(qh * QF, (qh + 1) * QF)
                ope = psp.tile([HD + 1, QF], f32, tag="o", name="o")
                opo = psp.tile([HD + 1, QF], f32, tag="o", name="o")
                prev = None
                for kcp in range(NT // 2):
                    spe = psp.tile([P, 2 * QF], f32, tag="s", name="s")
                    spo = psp.tile([P, 2 * QF], f32, tag="s", name="s")
                    for j in range(2):
                        ks_ = slice((2 * kcp + j) * P, (2 * kcp + j + 1) * P)
                        js = slice(j * QF, (j + 1) * QF)
                        nc.tensor.matmul(
                            spe[:, js], ke[:, ks_], qtile[:, qs],
                            start=True, stop=True)
                        nc.tensor.matmul(
                            spo[:, js], ko[:, ks_], qtile[:, qs],
                            start=True, stop=True)
                    ee = etp.tile([P, 2 * QF], bf16, tag="e", name="e")
                    eo = etp.tile([P, 2 * QF], bf16, tag="e", name="e")
                    nc.scalar.activation(ee[:], spe[:], AF.Exp, scale=0.125)
                    nc.scalar.activation(eo[:], spo[:], AF.Exp, scale=0.125)
                    # attn@V for the previous k-chunk pair overlaps this
                    # pair's exp latency on the PE
                    if prev is not None:
                        pee, peo, pk = prev
                        for j in range(2):
                            kc = 2 * pk + j
                            js = slice(j * QF, (j + 1) * QF)
                            nc.tensor.matmul(
                                ope[:], vst[kc][:, 2 * pi, :], pee[:, js],
                                start=(kc == 0), stop=False)
                            nc.tensor.matmul(
                                opo[:], vst[kc][:, 2 * pi + 1, :], peo[:, js],
                                start=(kc == 0), stop=False)
                    prev = (ee, eo, kcp)
                pee, peo, pk = prev
                for j in range(2):
                    kc = 2 * pk + j
                    js = slice(j * QF, (j + 1) * QF)
                    nc.tensor.matmul(
                        ope[:], vst[kc][:, 2 * pi, :], pee[:, js],
                        start=False, stop=(kc == NT - 1))
                    nc.tensor.matmul(
                        opo[:], vst[kc][:, 2 * pi + 1, :], peo[:, js],
                        start=False, stop=(kc == NT - 1))

                # evict PSUM immediately (quick DVE copies free the "o"
                # slots), then flush the reciprocal chain right away --
                # its gpsimd/DVE latency hides under the next attention
                # block. The last head-pair's o-projection trails one
                # q-block so flush(qh-1) has a full block to complete.
                osb_e = npool.tile([HD + 1, QF], f32, tag="osb", name="osb",
                                   bufs=6)
                nc.vector.tensor_copy(osb_e[:], ope[:])
                osb_o = npool.tile([HD + 1, QF], f32, tag="osb", name="osb",
                                   bufs=6)
                nc.vector.tensor_copy(osb_o[:], opo[:])
                pend.append((pi, qs, osb_e, osb_o,
                             pi == NPI - 1 and qh == NQF - 1))
                flush_one()
                if pi == NPI - 1 and qh > 0:
                    oproj(qh - 1)
            cur, nxt = nxt, None

        oproj_last()
    nc.compile()
    return nc


def _rope_tables(pos):
    """pos [n] -> cos/sin tables [128, n] bf16 (sign-folded sin)."""
    inv = ROPE_BASE ** (-np.arange(0, HD, 2, dtype=np.float64) / HD)
    fr = np.outer(pos.astype(np.float64), inv)          # [n, 32]
    c, s = np.cos(fr), np.sin(fr)
    cos64 = np.concatenate([c, c], axis=1).T            # [64, n]
    sinA = np.concatenate([s, -s], axis=1).T            # [64, n]
    ck = np.concatenate([cos64, cos64], axis=0).astype(BF16)
    sk = np.concatenate([sinA, sinA], axis=0).astype(BF16)
    return ck, sk


def _prep_wqk(w, g):
    """[D, D] weight -> bf16 [P, NPI, ND, P]: W.T columns for head group
    g, prearranged so each per-pi stationary DMA is one contiguous 2KB
    descriptor per partition."""
    wt = np.asarray(w, dtype=np.float32).T[:, g * DO:(g + 1) * DO]
    arr = wt.reshape(ND, P, NPI, P).transpose(1, 2, 0, 3)
    return np.ascontiguousarray(arr).astype(BF16)


def _prep_wv(w, g):
    """[D, D] weight -> bf16 [P, ND, DO] (prearranged W.T columns)."""
    wt = np.asarray(w, dtype=np.float32).T[:, g * DO:(g + 1) * DO]
    arr = wt.reshape(ND, P, DO).transpose(1, 0, 2)
    return np.ascontiguousarray(arr).astype(BF16)


def _prep_wo(w, g):
    """[D, D] weight -> bf16 [P, NPI, D]: W.T rows for head group g."""
    wt = np.asarray(w, dtype=np.float32).T[g * DO:(g + 1) * DO, :]
    arr = wt.reshape(NPI, P, D).transpose(1, 0, 2)
    return np.ascontiguousarray(arr).astype(BF16)


def _prep_b(b, g):
    return np.ascontiguousarray(
        np.asarray(b, dtype=np.float32)[None, g * DO:(g + 1) * DO]).astype(BF16)


def kernel(hidden_states, position_ids, Wq, bq, Wk, bk, Wv, bv, Wo):
    from concourse import bass_utils

    with_bias = bool(
        np.any(np.asarray(bq)) or np.any(np.asarray(bk)) or np.any(np.asarray(bv)))
    key = ("nc", with_bias)
    if key not in _cache:
        _cache[key] = _build_nc(with_bias)
    nc = _cache[key]

    hs = np.asarray(hidden_states, dtype=np.float32)
    pos = np.asarray(position_ids)
    wq = [_prep_wqk(Wq, g) for g in range(2)]
    wk = [_prep_wqk(Wk, g) for g in range(2)]
    wv = [_prep_wv(Wv, g) for g in range(2)]
    wo = [_prep_wo(Wo, g) for g in range(2)]
    bqs = [_prep_b(bq, g) for g in range(2)]
    bks = [_prep_b(bk, g) for g in range(2)]
    bvs = [_prep_b(bv, g) for g in range(2)]

    xts, tabs = [], []
    for b in range(B):
        xT = np.empty((D + 1, SK), dtype=np.float32)
        xT[:D] = hs[b].T
        xT[D] = 1.0
        xts.append(np.ascontiguousarray(xT).astype(BF16))
        tabs.append(_rope_tables(np.asarray(pos[b])))

    in_maps = []
    for core in range(NCORES):
        b, g = core // 2, core % 2
        ck, sk = tabs[b]
        in_maps.append({
            "xT": xts[b], "wqT": wq[g], "wkT": wk[g], "wvT": wv[g],
            "woT": wo[g], "wqb": bqs[g], "wkb": bks[g], "wvb": bvs[g],
            "cosk": ck, "sink": sk,
        })

    res = bass_utils.run_bass_kernel_spmd(
        nc, in_maps, core_ids=list(range(NCORES)), trace=TRACE, **TRACE_KW)
    LAST["exec_time_ns"] = res.exec_time_ns
    LAST["mean_exec_time_ns"] = res.mean_exec_time_ns
    LAST["trace"] = res.instructions_and_trace
    LAST["profile_json"] = res.profile_json

    outp_full = np.empty((B, S, D), dtype=np.float32)
    for b in range(B):
        outp_full[b] = (
            np.asarray(res.results[2 * b]["out"], dtype=np.float32)
            + np.asarray(res.results[2 * b + 1]["out"], dtype=np.float32))
    return outp_full


# revision 44
# speedup vs baseline: 1.3651x; 1.1824x over previous
"""Distributed Trainium2 Bass kernel for multi-head attention w/ RoPE.

Reference op (B=4, S=2048, D=1024, H=16, HD=64, fp32):
    q/k/v = hidden @ W{q,k,v}.T + b   (per-head reshape)
    q, k  = rope(q), rope(k)
    out   = softmax(q k^T / sqrt(HD)) v  @ Wo.T

Sharding v2: 8 cores = 4 batches x 2 head-groups (8 heads each). Every
core projects Q/K/V only for its own 512 features over the full 2048
tokens (no duplicated work anywhere -- PE row count is at the
theoretical floor of 786432 rows/core), runs attention for its 8 heads,
and o-projects its feature slice against the matching Wo rows. The two
half-outputs per batch are summed on the host (pure unshard add).

Single fused pipeline, fully transposed layout (features on partitions):
V projects first (natural layout, ones column appended so the softmax
denominator falls out of the attn@V matmul); then per head-pair: Q/K^T
projection chunks -> RoPE (DVE muls + a batched DMA partition band-swap
+ adds). K lands in TWO zero-padded stationary tiles (even head in rows
0:64 of ke, odd head in rows 64:128 of ko, other half zero via
parity-masked cos tables) so every scores matmul is a full 128-row
(128,128) PE tile against the full 128-row qtile moving operand --
avoiding the ~150ns PE reconfigure penalty that 64-row stationaries pay
on every row-size switch. Scores -> wide [128,1024] exp on ACT (scale
1/8 folded in, ACT does nothing else) -> attn@V interleaved one k-chunk
pair behind so PE fills ACT's exp latency. Normalization is
evicted-early (DVE copy frees PSUM), flushed one q-block late: exact
reciprocal runs 64-wide, gpsimd hops/broadcasts it, odd heads hop into
the o-proj operand via DMA. The o-projection for the last head-pair is
pipelined per q-block behind the final attention sweeps; output is
written bf16 and upcast host-side. All matmuls bf16, fp32 accumulation.
Nonzero biases ride an augmented K=1 contraction row (skipped when the
caller's biases are all zero).
"""

import sys

import numpy as np

try:  # concourse ships in the container; fall back to the staged repo
    import concourse.bass  # noqa: F401
except Exception:  # pragma: no cover
    sys.path.insert(0, "/opt/trn_rl_repo")

import ml_dtypes

B, S, D, H = 4, 2048, 1024, 16
HD = D // H                      # 64
P = 128
NCORES = 8
SK = S                           # 2048 tokens per core (q and k)
DO = 512                         # per-core head-group width (8 heads)
HC = 8                           # heads per core
ND = D // P                      # 8 feature contraction chunks
NPI = DO // P                    # 4 head-pair chunks
NT = SK // P                     # 16 key/token chunks
QF = 512                         # matmul moving width
NQF = SK // QF                   # 4 query blocks
ROPE_BASE = 10000.0
BF16 = ml_dtypes.bfloat16

TRACE = False                    # test harness flips this
TRACE_KW = {}
LAST = {}                        # exec_time_ns / trace path for test harness

_cache = {}


def _build_nc(with_bias):
    import concourse.bass as bass
    import concourse.mybir as mybir
    import concourse.tile as tile
    from concourse import bacc
    from contextlib import ExitStack

    f32 = mybir.dt.float32
    bf16 = mybir.dt.bfloat16
    AF = mybir.ActivationFunctionType
    PSUM = bass.MemorySpace.PSUM

    nc = bacc.Bacc(None)
    xT = nc.declare_dram_parameter("xT", [D + 1, SK], bf16, False)
    # weights are host-prearranged so every DMA is one fat contiguous
    # descriptor per partition (the natural W.T slices would shatter
    # into 1024 x 256B descriptors and clog all 16 DMA queues)
    wqT = nc.declare_dram_parameter("wqT", [P, NPI, ND, P], bf16, False)
    wkT = nc.declare_dram_parameter("wkT", [P, NPI, ND, P], bf16, False)
    wvT = nc.declare_dram_parameter("wvT", [P, ND, DO], bf16, False)
    woT = nc.declare_dram_parameter("woT", [P, NPI, D], bf16, False)
    wqb = nc.declare_dram_parameter("wqb", [1, DO], bf16, False)
    wkb = nc.declare_dram_parameter("wkb", [1, DO], bf16, False)
    wvb = nc.declare_dram_parameter("wvb", [1, DO], bf16, False)
    cosk = nc.declare_dram_parameter("cosk", [P, SK], bf16, False)
    sink = nc.declare_dram_parameter("sink", [P, SK], bf16, False)
    out = nc.declare_dram_parameter("out", [SK, D], bf16, True)

    with tile.TileContext(nc) as tc, ExitStack() as st:
        sb = st.enter_context(tc.tile_pool(name="sb", bufs=1))
        qk = st.enter_context(tc.tile_pool(name="qk", bufs=2))
        wp = st.enter_context(tc.tile_pool(name="wp", bufs=2))
        tp = st.enter_context(tc.tile_pool(name="tp", bufs=2))
        etp = st.enter_context(tc.tile_pool(name="et", bufs=8))
        npool = st.enter_context(tc.tile_pool(name="nrm", bufs=3))
        outp = st.enter_context(tc.tile_pool(name="ou", bufs=3))
        psp = st.enter_context(tc.tile_pool(name="ps", bufs=2, space=PSUM))

        vst = [sb.tile([P, HC, HD + 1], bf16, tag=f"v{t}", name=f"v{t}")
               for t in range(NT)]
        ones64 = sb.tile([1, HD], f32, tag="one64", name="one64")
        nc.vector.memset(ones64[:], 1.0)
        at = [sb.tile([P, SK], bf16, tag=f"at{i}", name=f"at{i}")
              for i in range(NPI)]

        # ---- loads (issue order = need order: pi0 weight slices first
        # so the projection chains can chase the x^T chunk DMAs) --------
        def load_wslice(wdram, wbdram, pi, wtag):
            ws = wp.tile([P, ND, P], bf16, tag=wtag, name=wtag)
            nc.sync.dma_start(out=ws[:], in_=wdram[:, pi, :, :])
            wb = None
            if with_bias:
                wb = wp.tile([1, P], bf16, tag=wtag + "b", name=wtag + "b")
                nc.sync.dma_start(out=wb[:], in_=wbdram[:, pi * P:(pi + 1) * P])
            return ws, wb

        wnext = (load_wslice(wqT, wqb, 0, "wq"), load_wslice(wkT, wkb, 0, "wk"))
        xs = [sb.tile([P, SK], bf16, tag=f"x{d}", name=f"x{d}")
              for d in range(ND)]
        for d_ in range(ND):
            nc.sync.dma_start(out=xs[d_][:], in_=xT[d_ * P:(d_ + 1) * P, :])
        if with_bias:
            xone = sb.tile([1, SK], bf16, tag="xone", name="xone")
            nc.sync.dma_start(out=xone[:], in_=xT[D:D + 1, :])
        # everything else queues on sync BEHIND x^T: the 16 DMA rings
        # round-robin all outstanding descriptors, so issuing these from
        # another queue would steal bandwidth from the critical-path x^T
        # chunks the first projection chains are chasing
        ck = sb.tile([P, SK], bf16, tag="ck", name="ck")
        sk_ = sb.tile([P, SK], bf16, tag="sk", name="sk")
        nc.sync.dma_start(out=ck[:], in_=cosk[:, :])
        nc.sync.dma_start(out=sk_[:], in_=sink[:, :])
        # zero the off-parity halves of the ke/ko pool buffers once (the
        # rope writes never touch them), instead of shipping masked cos
        # tables -- saves 1MB of critical-path DMA
        for _ in range(2):
            tke = qk.tile([P, SK], bf16, tag="ke", name="kez")
            nc.vector.memset(tke[HD:P, :], 0.0)
            tko = qk.tile([P, SK], bf16, tag="ko", name="koz")
            nc.vector.memset(tko[0:HD, :], 0.0)
        wv = wp.tile([P, ND, DO], bf16, tag="wv", name="wv", bufs=1)
        nc.sync.dma_start(out=wv[:], in_=wvT[:])
        if with_bias:
            wvbt = wp.tile([1, DO], bf16, tag="wvb", name="wvb", bufs=1)
            nc.sync.dma_start(out=wvbt[:], in_=wvb[:])
        wo = wp.tile([P, NPI, D], bf16, tag="wo", name="wo", bufs=1)
        nc.sync.dma_start(out=wo[:], in_=woT[:])

        def qk_proj(wsb, dst, dsto=None, dmaq=None):
            """dst = rope(W[pi-slice] @ x^T + b). Q path (dsto None):
            full-width writes into dst. K path: even head -> dst rows
            0:64 (rows 64:128 stay zero via the masked cos table), odd
            head -> dsto rows 64:128 -- zero-padded 128-row stationaries
            for the scores matmuls."""
            ws, wb = wsb
            t2 = tp.tile([P, SK], bf16, tag="t2", name="t2")
            t2s = tp.tile([P, SK], bf16, tag="t2s", name="t2s")
            for c in range(SK // QF):
                ps = psp.tile([P, QF], f32, tag="pp", name="pp")
                for d_ in range(ND):
                    nc.tensor.matmul(
                        ps[:], ws[:, d_, :], xs[d_][:, c * QF:(c + 1) * QF],
                        start=(d_ == 0), stop=(not with_bias and d_ == ND - 1))
                if with_bias:
                    nc.tensor.matmul(
                        ps[:], wb[:], xone[:, c * QF:(c + 1) * QF],
                        start=False, stop=True)
                cs = slice(c * QF, (c + 1) * QF)
                if dsto is None:
                    nc.vector.tensor_mul(dst[:, cs], ps[:], ck[:, cs])
                else:
                    nc.vector.tensor_mul(
                        dst[0:HD, cs], ps[0:HD, :], ck[0:HD, cs])
                    nc.vector.tensor_mul(
                        dsto[HD:P, cs], ps[HD:P, :], ck[HD:P, cs])
                nc.vector.tensor_mul(t2[:, cs], ps[:], sk_[:, cs])
                if c % 2 == 1:
                    # band swap d<->d+32 (pi0 rides the scalar DMA queue
                    # while sync drains the x^T loads; later pi use sync
                    # so swaps never queue behind exp issues) + add,
                    # batched over the finished 1024-wide half
                    hs_ = slice((c - 1) * QF, (c + 1) * QF)
                    for b0 in (0, 64):
                        dmaq.dma_start(
                            out=t2s[b0:b0 + 32, hs_], in_=t2[b0 + 32:b0 + 64, hs_])
                        dmaq.dma_start(
                            out=t2s[b0 + 32:b0 + 64, hs_], in_=t2[b0:b0 + 32, hs_])
                    if dsto is None:
                        nc.vector.tensor_add(
                            dst[:, hs_], dst[:, hs_], t2s[:, hs_])
                    else:
                        nc.vector.tensor_add(
                            dst[0:HD, hs_], dst[0:HD, hs_], t2s[0:HD, hs_])
                        nc.vector.tensor_add(
                            dsto[HD:P, hs_], dsto[HD:P, hs_], t2s[HD:P, hs_])

        def v_proj():
            # V projection (natural layout, x^T stationary)
            for t_ in range(NT):
                ps = psp.tile([P, DO], f32, tag="pp", name="pp")
                for d_ in range(ND):
                    nc.tensor.matmul(
                        ps[:], xs[d_][:, t_ * P:(t_ + 1) * P], wv[:, d_, :],
                        start=(d_ == 0), stop=(not with_bias and d_ == ND - 1))
                if with_bias:
                    nc.tensor.matmul(
                        ps[:], xone[:, t_ * P:(t_ + 1) * P], wvbt[:],
                        start=False, stop=True)
                nc.vector.tensor_copy(
                    vst[t_][:, :, 0:HD],
                    ps[:].rearrange("p (h d) -> p h d", d=HD))
                nc.vector.memset(vst[t_][:, :, HD:HD + 1], 1.0)

        # ---- fused per-head-pair projection + attention ----------------
        pend = []

        def flush_one():
            # normalize in SBUF: exact reciprocal spread 64-wide (~0.3us
            # not 3.3us single-lane), DMA-hop to p0, partition-
            # broadcast, multiply into the o-proj operand. The even
            # head's hops ride the sync queue so the two parity chains
            # drain in parallel (gpsimd serializes its own hops)
            pi, qqs, osb_e, osb_o, last = pend.pop(0)
            for par, osb, dq in ((0, osb_e, nc.sync), (1, osb_o, nc.gpsimd)):
                smr = npool.tile([HD, 8], f32, tag="smr", name="smr")
                dq.dma_start(out=smr[:], in_=osb[HD:HD + 1, :])
                rcs = npool.tile([HD, 8], f32, tag="rcs", name="rcs")
                nc.vector.reciprocal(rcs[:], smr[:])
                rc = npool.tile([1, QF], f32, tag="rc", name="rc")
                dq.dma_start(out=rc[:], in_=rcs[:])
                if last:
                    # the final flush is latency-exposed: broadcast on
                    # the (idle) PE via a ones stationary instead of the
                    # ~1.1us gpsimd PartitionBroadcast
                    bcp = psp.tile([HD + 1, QF], f32, tag="o", name="bcp")
                    nc.tensor.matmul(bcp[0:HD, :], ones64[:], rc[:],
                                     start=True, stop=True)
                    bc = bcp[0:HD, :]
                else:
                    bct = npool.tile([HD, QF], f32, tag="bc", name="bc")
                    nc.gpsimd.partition_broadcast(bct[:], rc[:])
                    bc = bct[:]
                if par == 0:
                    nc.vector.tensor_mul(
                        at[pi][0:HD, qqs], osb[0:HD, :], bc)
                else:
                    # odd heads land at partition base 64; a pure-SBUF
                    # base-shifted DVE write corrupts, so write at base
                    # 0 and DMA-hop into place (sync queue: it gates the
                    # last head-pair's o-projection)
                    atm = npool.tile([HD, QF], bf16, tag="atm", name="atm")
                    nc.vector.tensor_mul(atm[:], osb[0:HD, :], bc)
                    nc.sync.dma_start(out=at[pi][HD:P, qqs], in_=atm[:])

        def _oproj_close(view, qa, oh, act=False):
            # the final q block evicts via ACT (its exp work is done and
            # DVE still has flush multiplies in flight); earlier blocks
            # stay on DVE since ACT is still running the next attention
            # block's exps
            ob = outp.tile([P, QF], bf16, tag="ob", name="ob")
            if act:
                nc.scalar.activation(ob[:], view, AF.Copy)
            else:
                nc.vector.tensor_copy(ob[:], view)
            dq = nc.scalar if (qa + oh) % 2 == 0 else nc.sync
            dq.dma_start(
                out=out[qa * P:(qa + 1) * P, oh * QF:(oh + 1) * QF],
                in_=ob[:])

        def oproj(qh):
            # o-projection for one 512-wide q block; consumes the
            # transposed at[] tiles directly, writes bf16
            for qc in range(QF // P):
                qa = qh * (QF // P) + qc
                for oh in range(2):
                    ps = psp.tile([P, QF], f32, tag="pp", name="pp")
                    for f in range(NPI):
                        nc.tensor.matmul(
                            ps[:], at[f][:, qa * P:(qa + 1) * P],
                            wo[:, f, oh * QF:(oh + 1) * QF],
                            start=(f == 0), stop=(f == NPI - 1))
                    _oproj_close(ps[:], qa, oh)

        def oproj_last():
            # final q block: the f<3 partials of the first chains are
            # emitted open (no stop) so the PE chews them while the last
            # flush chain drains; only the f=3 matmuls wait on at[3].
            # Free "s"-tag banks host 4 of the early chains
            slots = [(qh_ * 0 + (NQF - 1) * (QF // P) + qc, oh)
                     for qc in range(QF // P) for oh in range(2)
                     for qh_ in (0,)]
            views = []
            for i, (qa, oh) in enumerate(slots[:6]):
                if i < 2:
                    ps = psp.tile([P, QF], f32, tag="pp", name="pp")
                    views.append(ps[:])
                else:
                    if i % 2 == 0:
                        stile = psp.tile([P, 2 * QF], f32, tag="s", name="s")
                    views.append(stile[:, (i % 2) * QF:(i % 2 + 1) * QF])
                for f in range(NPI - 1):
                    nc.tensor.matmul(
                        views[i], at[f][:, qa * P:(qa + 1) * P],
                        wo[:, f, oh * QF:(oh + 1) * QF],
                        start=(f == 0), stop=False, skip_group_check=True)
            for i, (qa, oh) in enumerate(slots[:6]):
                nc.tensor.matmul(
                    views[i], at[NPI - 1][:, qa * P:(qa + 1) * P],
                    wo[:, NPI - 1, oh * QF:(oh + 1) * QF],
                    start=False, stop=True, skip_group_check=True)
                _oproj_close(views[i], qa, oh)
            for qa, oh in slots[6:]:
                ps = psp.tile([P, QF], f32, tag="pp", name="pp")
                for f in range(NPI):
                    nc.tensor.matmul(
                        ps[:], at[f][:, qa * P:(qa + 1) * P],
                        wo[:, f, oh * QF:(oh + 1) * QF],
                        start=(f == 0), stop=(f == NPI - 1))
                _oproj_close(ps[:], qa, oh)

        def do_qk(wsb_pair, dmaq):
            qtile = qk.tile([P, SK], bf16, tag="qt", name="qt")
            qk_proj(wsb_pair[0], qtile, dmaq=dmaq)
            ke = qk.tile([P, SK], bf16, tag="ke", name="ke")
            ko = qk.tile([P, SK], bf16, tag="ko", name="ko")
            qk_proj(wsb_pair[1], ke, ko, dmaq=dmaq)
            return qtile, ke, ko

        # software pipeline: the next head-pair's Q/K projection (PE
        # chains + DVE rope + swap DMAs) is emitted before the CURRENT
        # pair's last attention block, so its rope pipeline drains while
        # the PE is still busy -- no dead time at head-pair boundaries
        cur = do_qk(wnext, nc.scalar)
        v_proj()
        nxt = None
        wnext = (load_wslice(wqT, wqb, 1, "wq"), load_wslice(wkT, wkb, 1, "wk"))
        for pi in range(NPI):
            qtile, ke, ko = cur

            for qh in range(NQF):
                if qh == NQF - 1 and pi + 1 < NPI:
                    nxt = do_qk(wnext, nc.sync)
                    if pi + 2 < NPI:
                        wnext = (load_wslice(wqT, wqb, pi + 2, "wq"),
                                 load_wslice(wkT, wkb, pi + 2, "wk"))
                qs = slice(qh * QF, (qh + 1) * QF)
                ope = psp.tile([HD + 1, QF], f32, tag="o", name="o")
                opo = psp.tile([HD + 1, QF], f32, tag="o", name="o")
                prev = None
                for kcp in range(NT // 2):
                    spe = psp.tile([P, 2 * QF], f32, tag="s", name="s")
                    spo = psp.tile([P, 2 * QF], f32, tag="s", name="s")
                    for j in range(2):
                        ks_ = slice((2 * kcp + j) * P, (2 * kcp + j + 1) * P)
                        js = slice(j * QF, (j + 1) * QF)
                        nc.tensor.matmul(
                            spe[:, js], ke[:, ks_], qtile[:, qs],
                            start=True, stop=True)
                        nc.tensor.matmul(
                            spo[:, js], ko[:, ks_], qtile[:, qs],
                            start=True, stop=True)
                    ee = etp.tile([P, 2 * QF], bf16, tag="e", name="e")
                    eo = etp.tile([P, 2 * QF], bf16, tag="e", name="e")
                    nc.scalar.activation(ee[:], spe[:], AF.Exp, scale=0.125)
                    nc.scalar.activation(eo[:], spo[:], AF.Exp, scale=0.125)
                    # attn@V for the previous k-chunk pair overlaps this
                    # pair's exp latency on the PE
                    if prev is not None:
                        pee, peo, pk = prev
                        for j in range(2):
                            kc = 2 * pk + j
                            js = slice(j * QF, (j + 1) * QF)
                            nc.tensor.matmul(
                                ope[:], vst[kc][:, 2 * pi, :], pee[:, js],
                                start=(kc == 0), stop=False)
                            nc.tensor.matmul(
                                opo[:], vst[kc][:, 2 * pi + 1, :], peo[:, js],
                                start=(kc == 0), stop=False)
                    prev = (ee, eo, kcp)
                pee, peo, pk = prev
                for j in range(2):
                    kc = 2 * pk + j
                    js = slice(j * QF, (j + 1) * QF)
                    nc.tensor.matmul(
                        ope[:], vst[kc][:, 2 * pi, :], pee[:, js],
                        start=False, stop=(kc == NT - 1))
                    nc.tensor.matmul(
                        opo[:], vst[kc][:, 2 * pi + 1, :], peo[:, js],
                        start=False, stop=(kc == NT - 1))

                # evict PSUM immediately (quick DVE copies free the "o"
                # slots), then flush the reciprocal chain right away --
                # its gpsimd/DVE latency hides under the next attention
                # block. The last head-pair's o-projection trails one
                # q-block so flush(qh-1) has a full block to complete.
                osb_e = npool.tile([HD + 1, QF], f32, tag="osb", name="osb",
                                   bufs=8)
                nc.vector.tensor_copy(osb_e[:], ope[:])
                osb_o = npool.tile([HD + 1, QF], f32, tag="osb", name="osb",
                                   bufs=8)
                nc.vector.tensor_copy(osb_o[:], opo[:])
                pend.append((pi, qs, osb_e, osb_o,
                             pi == NPI - 1 and qh == NQF - 1))
                flush_one()
                if pi == NPI - 1 and qh > 0:
                    oproj(qh - 1)
            cur, nxt = nxt, None

        oproj_last()
    nc.compile()
    return nc


def _rope_tables(pos):
    """pos [n] -> cos/sin tables [128, n] bf16 (sign-folded sin)."""
    inv = ROPE_BASE ** (-np.arange(0, HD, 2, dtype=np.float64) / HD)
    fr = np.outer(pos.astype(np.float64), inv)          # [n, 32]
    c, s = np.cos(fr), np.sin(fr)
    cos64 = np.concatenate([c, c], axis=1).T            # [64, n]
    sinA = np.concatenate([s, -s], axis=1).T            # [64, n]
    ck = np.concatenate([cos64, cos64], axis=0).astype(BF16)
    sk = np.concatenate([sinA, sinA], axis=0).astype(BF16)
    return ck, sk


def _prep_wqk(w, g):
    """[D, D] weight -> bf16 [P, NPI, ND, P]: W.T columns for head group
    g, prearranged so each per-pi stationary DMA is one contiguous 2KB
    descriptor per partition."""
    wt = np.asarray(w, dtype=np.float32).T[:, g * DO:(g + 1) * DO]
    arr = wt.reshape(ND, P, NPI, P).transpose(1, 2, 0, 3)
    return np.ascontiguousarray(arr).astype(BF16)


def _prep_wv(w, g):
    """[D, D] weight -> bf16 [P, ND, DO] (prearranged W.T columns)."""
    wt = np.asarray(w, dtype=np.float32).T[:, g * DO:(g + 1) * DO]
    arr = wt.reshape(ND, P, DO).transpose(1, 0, 2)
    return np.ascontiguousarray(arr).astype(BF16)


def _prep_wo(w, g):
    """[D, D] weight -> bf16 [P, NPI, D]: W.T rows for head group g."""
    wt = np.asarray(w, dtype=np.float32).T[g * DO:(g + 1) * DO, :]
    arr = wt.reshape(NPI, P, D).transpose(1, 0, 2)
    return np.ascontiguousarray(arr).astype(BF16)


def _prep_b(b, g):
    return np.ascontiguousarray(
        np.asarray(b, dtype=np.float32)[None, g * DO:(g + 1) * DO]).astype(BF16)


def kernel(hidden_states, position_ids, Wq, bq, Wk, bk, Wv, bv, Wo):
    from concourse import bass_utils

    with_bias = bool(
        np.any(np.asarray(bq)) or np.any(np.asarray(bk)) or np.any(np.asarray(bv)))
    key = ("nc", with_bias)
    if key not in _cache:
        _cache[key] = _build_nc(with_bias)
    nc = _cache[key]

    hs = np.asarray(hidden_states, dtype=np.float32)
    pos = np.asarray(position_ids)
    wq = [_prep_wqk(Wq, g) for g in range(2)]
    wk = [_prep_wqk(Wk, g) for g in range(2)]
    wv = [_prep_wv(Wv, g) for g in range(2)]
    wo = [_prep_wo(Wo, g) for g in range(2)]
    bqs = [_prep_b(bq, g) for g in range(2)]
    bks = [_prep_b(bk, g) for g in range(2)]
    bvs = [_prep_b(bv, g) for g in range(2)]

    xts, tabs = [], []
    for b in range(B):
        xT = np.empty((D + 1, SK), dtype=np.float32)
        xT[:D] = hs[b].T
        xT[D] = 1.0
        xts.append(np.ascontiguousarray(xT).astype(BF16))
        tabs.append(_rope_tables(np.asarray(pos[b])))

    in_maps = []
    for core in range(NCORES):
        b, g = core // 2, core % 2
        ck, sk = tabs[b]
        in_maps.append({
            "xT": xts[b], "wqT": wq[g], "wkT": wk[g], "wvT": wv[g],
            "woT": wo[g], "wqb": bqs[g], "wkb": bks[g], "wvb": bvs[g],
            "cosk": ck, "sink": sk,
        })

    res = bass_utils.run_bass_kernel_spmd(
        nc, in_maps, core_ids=list(range(NCORES)), trace=TRACE, **TRACE_KW)
    LAST["exec_time_ns"] = res.exec_time_ns
    LAST["mean_exec_time_ns"] = res.mean_exec_time_ns
    LAST["trace"] = res.instructions_and_trace
    LAST["profile_json"] = res.profile_json

    outp_full = np.empty((B, S, D), dtype=np.float32)
    for b in range(B):
        outp_full[b] = (
            np.asarray(res.results[2 * b]["out"], dtype=np.float32)
            + np.asarray(res.results[2 * b + 1]["out"], dtype=np.float32))
    return outp_full


# revision 45
# speedup vs baseline: 1.3674x; 1.0017x over previous
"""Distributed Trainium2 Bass kernel for multi-head attention w/ RoPE.

Reference op (B=4, S=2048, D=1024, H=16, HD=64, fp32):
    q/k/v = hidden @ W{q,k,v}.T + b   (per-head reshape)
    q, k  = rope(q), rope(k)
    out   = softmax(q k^T / sqrt(HD)) v  @ Wo.T

Sharding v2: 8 cores = 4 batches x 2 head-groups (8 heads each). Every
core projects Q/K/V only for its own 512 features over the full 2048
tokens (no duplicated work anywhere -- PE row count is at the
theoretical floor of 786432 rows/core), runs attention for its 8 heads,
and o-projects its feature slice against the matching Wo rows. The two
half-outputs per batch are summed on the host (pure unshard add).

Single fused pipeline, fully transposed layout (features on partitions):
V projects first (natural layout, ones column appended so the softmax
denominator falls out of the attn@V matmul); then per head-pair: Q/K^T
projection chunks -> RoPE (DVE muls + a batched DMA partition band-swap
+ adds). K lands in TWO zero-padded stationary tiles (even head in rows
0:64 of ke, odd head in rows 64:128 of ko, other half zero via
parity-masked cos tables) so every scores matmul is a full 128-row
(128,128) PE tile against the full 128-row qtile moving operand --
avoiding the ~150ns PE reconfigure penalty that 64-row stationaries pay
on every row-size switch. Scores -> wide [128,1024] exp on ACT (scale
1/8 folded in, ACT does nothing else) -> attn@V interleaved one k-chunk
pair behind so PE fills ACT's exp latency. Normalization is
evicted-early (DVE copy frees PSUM), flushed one q-block late: exact
reciprocal runs 64-wide, gpsimd hops/broadcasts it, odd heads hop into
the o-proj operand via DMA. The o-projection for the last head-pair is
pipelined per q-block behind the final attention sweeps; output is
written bf16 and upcast host-side. All matmuls bf16, fp32 accumulation.
Nonzero biases ride an augmented K=1 contraction row (skipped when the
caller's biases are all zero).
"""

import sys

import numpy as np

try:  # concourse ships in the container; fall back to the staged repo
    import concourse.bass  # noqa: F401
except Exception:  # pragma: no cover
    sys.path.insert(0, "/opt/trn_rl_repo")

import ml_dtypes

B, S, D, H = 4, 2048, 1024, 16
HD = D // H                      # 64
P = 128
NCORES = 8
SK = S                           # 2048 tokens per core (q and k)
DO = 512                         # per-core head-group width (8 heads)
HC = 8                           # heads per core
ND = D // P                      # 8 feature contraction chunks
NPI = DO // P                    # 4 head-pair chunks
NT = SK // P                     # 16 key/token chunks
QF = 512                         # matmul moving width
NQF = SK // QF                   # 4 query blocks
ROPE_BASE = 10000.0
BF16 = ml_dtypes.bfloat16

TRACE = False                    # test harness flips this
TRACE_KW = {}
LAST = {}                        # exec_time_ns / trace path for test harness

_cache = {}


def _build_nc(with_bias):
    import concourse.bass as bass
    import concourse.mybir as mybir
    import concourse.tile as tile
    from concourse import bacc
    from contextlib import ExitStack

    f32 = mybir.dt.float32
    bf16 = mybir.dt.bfloat16
    AF = mybir.ActivationFunctionType
    PSUM = bass.MemorySpace.PSUM

    nc = bacc.Bacc(None)
    xT = nc.declare_dram_parameter("xT", [D + 1, SK], bf16, False)
    # weights are host-prearranged so every DMA is one fat contiguous
    # descriptor per partition (the natural W.T slices would shatter
    # into 1024 x 256B descriptors and clog all 16 DMA queues)
    wqT = nc.declare_dram_parameter("wqT", [P, NPI, ND, P], bf16, False)
    wkT = nc.declare_dram_parameter("wkT", [P, NPI, ND, P], bf16, False)
    wvT = nc.declare_dram_parameter("wvT", [P, ND, DO], bf16, False)
    woT = nc.declare_dram_parameter("woT", [P, NPI, D], bf16, False)
    wqb = nc.declare_dram_parameter("wqb", [1, DO], bf16, False)
    wkb = nc.declare_dram_parameter("wkb", [1, DO], bf16, False)
    wvb = nc.declare_dram_parameter("wvb", [1, DO], bf16, False)
    cosk = nc.declare_dram_parameter("cosk", [P, SK], bf16, False)
    sink = nc.declare_dram_parameter("sink", [P, SK], bf16, False)
    out = nc.declare_dram_parameter("out", [SK, D], bf16, True)

    with tile.TileContext(nc) as tc, ExitStack() as st:
        sb = st.enter_context(tc.tile_pool(name="sb", bufs=1))
        qk = st.enter_context(tc.tile_pool(name="qk", bufs=2))
        wp = st.enter_context(tc.tile_pool(name="wp", bufs=2))
        tp = st.enter_context(tc.tile_pool(name="tp", bufs=2))
        etp = st.enter_context(tc.tile_pool(name="et", bufs=8))
        npool = st.enter_context(tc.tile_pool(name="nrm", bufs=3))
        outp = st.enter_context(tc.tile_pool(name="ou", bufs=3))
        psp = st.enter_context(tc.tile_pool(name="ps", bufs=2, space=PSUM))

        vst = [sb.tile([P, HC, HD + 1], bf16, tag=f"v{t}", name=f"v{t}")
               for t in range(NT)]
        ones64 = sb.tile([1, HD], f32, tag="one64", name="one64")
        nc.vector.memset(ones64[:], 1.0)
        at = [sb.tile([P, SK], bf16, tag=f"at{i}", name=f"at{i}")
              for i in range(NPI)]

        # ---- loads (issue order = need order: pi0 weight slices first
        # so the projection chains can chase the x^T chunk DMAs) --------
        def load_wslice(wdram, wbdram, pi, wtag):
            ws = wp.tile([P, ND, P], bf16, tag=wtag, name=wtag)
            nc.sync.dma_start(out=ws[:], in_=wdram[:, pi, :, :])
            wb = None
            if with_bias:
                wb = wp.tile([1, P], bf16, tag=wtag + "b", name=wtag + "b")
                nc.sync.dma_start(out=wb[:], in_=wbdram[:, pi * P:(pi + 1) * P])
            return ws, wb

        wnext = (load_wslice(wqT, wqb, 0, "wq"), load_wslice(wkT, wkb, 0, "wk"))
        # the first projection chain needs ALL eight x^T chunks, and a
        # dma_start issue occupies its sequencer ~700ns -- alternate the
        # issues across sync and scalar so the last chunk is in flight
        # ~4us sooner
        xs = [sb.tile([P, SK], bf16, tag=f"x{d}", name=f"x{d}")
              for d in range(ND)]
        for d_ in range(ND):
            dq = nc.sync if d_ % 2 == 0 else nc.scalar
            dq.dma_start(out=xs[d_][:], in_=xT[d_ * P:(d_ + 1) * P, :])
        if with_bias:
            xone = sb.tile([1, SK], bf16, tag="xone", name="xone")
            nc.sync.dma_start(out=xone[:], in_=xT[D:D + 1, :])
        # everything else queues on sync BEHIND x^T: the 16 DMA rings
        # round-robin all outstanding descriptors, so issuing these from
        # another queue would steal bandwidth from the critical-path x^T
        # chunks the first projection chains are chasing
        ck = sb.tile([P, SK], bf16, tag="ck", name="ck")
        sk_ = sb.tile([P, SK], bf16, tag="sk", name="sk")
        nc.scalar.dma_start(out=ck[:], in_=cosk[:, :])
        nc.scalar.dma_start(out=sk_[:], in_=sink[:, :])
        # zero the off-parity halves of the ke/ko pool buffers once (the
        # rope writes never touch them), instead of shipping masked cos
        # tables -- saves 1MB of critical-path DMA
        for _ in range(2):
            tke = qk.tile([P, SK], bf16, tag="ke", name="kez")
            nc.vector.memset(tke[HD:P, :], 0.0)
            tko = qk.tile([P, SK], bf16, tag="ko", name="koz")
            nc.vector.memset(tko[0:HD, :], 0.0)
        wv = wp.tile([P, ND, DO], bf16, tag="wv", name="wv", bufs=1)
        nc.sync.dma_start(out=wv[:], in_=wvT[:])
        if with_bias:
            wvbt = wp.tile([1, DO], bf16, tag="wvb", name="wvb", bufs=1)
            nc.sync.dma_start(out=wvbt[:], in_=wvb[:])
        wo = wp.tile([P, NPI, D], bf16, tag="wo", name="wo", bufs=1)
        nc.sync.dma_start(out=wo[:], in_=woT[:])

        def qk_proj(wsb, dst, dsto=None, dmaq=None):
            """dst = rope(W[pi-slice] @ x^T + b). Q path (dsto None):
            full-width writes into dst. K path: even head -> dst rows
            0:64 (rows 64:128 stay zero via the masked cos table), odd
            head -> dsto rows 64:128 -- zero-padded 128-row stationaries
            for the scores matmuls."""
            ws, wb = wsb
            t2 = tp.tile([P, SK], bf16, tag="t2", name="t2")
            t2s = tp.tile([P, SK], bf16, tag="t2s", name="t2s")
            for c in range(SK // QF):
                ps = psp.tile([P, QF], f32, tag="pp", name="pp")
                for d_ in range(ND):
                    nc.tensor.matmul(
                        ps[:], ws[:, d_, :], xs[d_][:, c * QF:(c + 1) * QF],
                        start=(d_ == 0), stop=(not with_bias and d_ == ND - 1))
                if with_bias:
                    nc.tensor.matmul(
                        ps[:], wb[:], xone[:, c * QF:(c + 1) * QF],
                        start=False, stop=True)
                cs = slice(c * QF, (c + 1) * QF)
                if dsto is None:
                    nc.vector.tensor_mul(dst[:, cs], ps[:], ck[:, cs])
                else:
                    nc.vector.tensor_mul(
                        dst[0:HD, cs], ps[0:HD, :], ck[0:HD, cs])
                    nc.vector.tensor_mul(
                        dsto[HD:P, cs], ps[HD:P, :], ck[HD:P, cs])
                nc.vector.tensor_mul(t2[:, cs], ps[:], sk_[:, cs])
                if c % 2 == 1:
                    # band swap d<->d+32 (pi0 rides the scalar DMA queue
                    # while sync drains the x^T loads; later pi use sync
                    # so swaps never queue behind exp issues) + add,
                    # batched over the finished 1024-wide half
                    hs_ = slice((c - 1) * QF, (c + 1) * QF)
                    for b0 in (0, 64):
                        dmaq.dma_start(
                            out=t2s[b0:b0 + 32, hs_], in_=t2[b0 + 32:b0 + 64, hs_])
                        dmaq.dma_start(
                            out=t2s[b0 + 32:b0 + 64, hs_], in_=t2[b0:b0 + 32, hs_])
                    if dsto is None:
                        nc.vector.tensor_add(
                            dst[:, hs_], dst[:, hs_], t2s[:, hs_])
                    else:
                        nc.vector.tensor_add(
                            dst[0:HD, hs_], dst[0:HD, hs_], t2s[0:HD, hs_])
                        nc.vector.tensor_add(
                            dsto[HD:P, hs_], dsto[HD:P, hs_], t2s[HD:P, hs_])

        def v_proj():
            # V projection (natural layout, x^T stationary)
            for t_ in range(NT):
                ps = psp.tile([P, DO], f32, tag="pp", name="pp")
                for d_ in range(ND):
                    nc.tensor.matmul(
                        ps[:], xs[d_][:, t_ * P:(t_ + 1) * P], wv[:, d_, :],
                        start=(d_ == 0), stop=(not with_bias and d_ == ND - 1))
                if with_bias:
                    nc.tensor.matmul(
                        ps[:], xone[:, t_ * P:(t_ + 1) * P], wvbt[:],
                        start=False, stop=True)
                nc.vector.tensor_copy(
                    vst[t_][:, :, 0:HD],
                    ps[:].rearrange("p (h d) -> p h d", d=HD))
                nc.vector.memset(vst[t_][:, :, HD:HD + 1], 1.0)

        # ---- fused per-head-pair projection + attention ----------------
        pend = []

        def flush_one():
            # normalize in SBUF: exact reciprocal spread 64-wide (~0.3us
            # not 3.3us single-lane), DMA-hop to p0, partition-
            # broadcast, multiply into the o-proj operand. The even
            # head's hops ride the sync queue so the two parity chains
            # drain in parallel (gpsimd serializes its own hops)
            pi, qqs, osb_e, osb_o, last = pend.pop(0)
            for par, osb, dq in ((0, osb_e, nc.sync), (1, osb_o, nc.gpsimd)):
                smr = npool.tile([HD, 8], f32, tag="smr", name="smr")
                dq.dma_start(out=smr[:], in_=osb[HD:HD + 1, :])
                rcs = npool.tile([HD, 8], f32, tag="rcs", name="rcs")
                nc.vector.reciprocal(rcs[:], smr[:])
                rc = npool.tile([1, QF], f32, tag="rc", name="rc")
                dq.dma_start(out=rc[:], in_=rcs[:])
                if last:
                    # the final flush is latency-exposed: broadcast on
                    # the (idle) PE via a ones stationary instead of the
                    # ~1.1us gpsimd PartitionBroadcast
                    bcp = psp.tile([HD + 1, QF], f32, tag="o", name="bcp")
                    nc.tensor.matmul(bcp[0:HD, :], ones64[:], rc[:],
                                     start=True, stop=True)
                    bc = bcp[0:HD, :]
                else:
                    bct = npool.tile([HD, QF], f32, tag="bc", name="bc")
                    nc.gpsimd.partition_broadcast(bct[:], rc[:])
                    bc = bct[:]
                if par == 0:
                    nc.vector.tensor_mul(
                        at[pi][0:HD, qqs], osb[0:HD, :], bc)
                else:
                    # odd heads land at partition base 64; a pure-SBUF
                    # base-shifted DVE write corrupts, so write at base
                    # 0 and DMA-hop into place (sync queue: it gates the
                    # last head-pair's o-projection)
                    atm = npool.tile([HD, QF], bf16, tag="atm", name="atm")
                    nc.vector.tensor_mul(atm[:], osb[0:HD, :], bc)
                    nc.sync.dma_start(out=at[pi][HD:P, qqs], in_=atm[:])

        def _oproj_close(view, qa, oh, act=False):
            # the final q block evicts via ACT (its exp work is done and
            # DVE still has flush multiplies in flight); earlier blocks
            # stay on DVE since ACT is still running the next attention
            # block's exps
            ob = outp.tile([P, QF], bf16, tag="ob", name="ob")
            if act:
                nc.scalar.activation(ob[:], view, AF.Copy)
            else:
                nc.vector.tensor_copy(ob[:], view)
            dq = nc.scalar if (qa + oh) % 2 == 0 else nc.sync
            dq.dma_start(
                out=out[qa * P:(qa + 1) * P, oh * QF:(oh + 1) * QF],
                in_=ob[:])

        def oproj(qh):
            # o-projection for one 512-wide q block; consumes the
            # transposed at[] tiles directly, writes bf16
            for qc in range(QF // P):
                qa = qh * (QF // P) + qc
                for oh in range(2):
                    ps = psp.tile([P, QF], f32, tag="pp", name="pp")
                    for f in range(NPI):
                        nc.tensor.matmul(
                            ps[:], at[f][:, qa * P:(qa + 1) * P],
                            wo[:, f, oh * QF:(oh + 1) * QF],
                            start=(f == 0), stop=(f == NPI - 1))
                    _oproj_close(ps[:], qa, oh)

        def oproj_last():
            # final q block: the f<3 partials of the first chains are
            # emitted open (no stop) so the PE chews them while the last
            # flush chain drains; only the f=3 matmuls wait on at[3].
            # Free "s"-tag banks host 4 of the early chains
            slots = [(qh_ * 0 + (NQF - 1) * (QF // P) + qc, oh)
                     for qc in range(QF // P) for oh in range(2)
                     for qh_ in (0,)]
            views = []
            for i, (qa, oh) in enumerate(slots[:6]):
                if i < 2:
                    ps = psp.tile([P, QF], f32, tag="pp", name="pp")
                    views.append(ps[:])
                else:
                    if i % 2 == 0:
                        stile = psp.tile([P, 2 * QF], f32, tag="s", name="s")
                    views.append(stile[:, (i % 2) * QF:(i % 2 + 1) * QF])
                for f in range(NPI - 1):
                    nc.tensor.matmul(
                        views[i], at[f][:, qa * P:(qa + 1) * P],
                        wo[:, f, oh * QF:(oh + 1) * QF],
                        start=(f == 0), stop=False, skip_group_check=True)
            for i, (qa, oh) in enumerate(slots[:6]):
                nc.tensor.matmul(
                    views[i], at[NPI - 1][:, qa * P:(qa + 1) * P],
                    wo[:, NPI - 1, oh * QF:(oh + 1) * QF],
                    start=False, stop=True, skip_group_check=True)
                _oproj_close(views[i], qa, oh)
            for qa, oh in slots[6:]:
                ps = psp.tile([P, QF], f32, tag="pp", name="pp")
                for f in range(NPI):
                    nc.tensor.matmul(
                        ps[:], at[f][:, qa * P:(qa + 1) * P],
                        wo[:, f, oh * QF:(oh + 1) * QF],
                        start=(f == 0), stop=(f == NPI - 1))
                _oproj_close(ps[:], qa, oh)

        def do_qk(wsb_pair, dmaq):
            qtile = qk.tile([P, SK], bf16, tag="qt", name="qt")
            qk_proj(wsb_pair[0], qtile, dmaq=dmaq)
            ke = qk.tile([P, SK], bf16, tag="ke", name="ke")
            ko = qk.tile([P, SK], bf16, tag="ko", name="ko")
            qk_proj(wsb_pair[1], ke, ko, dmaq=dmaq)
            return qtile, ke, ko

        # software pipeline: the next head-pair's Q/K projection (PE
        # chains + DVE rope + swap DMAs) is emitted before the CURRENT
        # pair's last attention block, so its rope pipeline drains while
        # the PE is still busy -- no dead time at head-pair boundaries
        cur = do_qk(wnext, nc.scalar)
        v_proj()
        nxt = None
        wnext = (load_wslice(wqT, wqb, 1, "wq"), load_wslice(wkT, wkb, 1, "wk"))
        for pi in range(NPI):
            qtile, ke, ko = cur

            for qh in range(NQF):
                if qh == NQF - 1 and pi + 1 < NPI:
                    nxt = do_qk(wnext, nc.sync)
                    if pi + 2 < NPI:
                        wnext = (load_wslice(wqT, wqb, pi + 2, "wq"),
                                 load_wslice(wkT, wkb, pi + 2, "wk"))
                qs = slice(qh * QF, (qh + 1) * QF)
                ope = psp.tile([HD + 1, QF], f32, tag="o", name="o")
                opo = psp.tile([HD + 1, QF], f32, tag="o", name="o")
                prev = None
                for kcp in range(NT // 2):
                    spe = psp.tile([P, 2 * QF], f32, tag="s", name="s")
                    spo = psp.tile([P, 2 * QF], f32, tag="s", name="s")
                    for j in range(2):
                        ks_ = slice((2 * kcp + j) * P, (2 * kcp + j + 1) * P)
                        js = slice(j * QF, (j + 1) * QF)
                        nc.tensor.matmul(
                            spe[:, js], ke[:, ks_], qtile[:, qs],
                            start=True, stop=True)
                        nc.tensor.matmul(
                            spo[:, js], ko[:, ks_], qtile[:, qs],
                            start=True, stop=True)
                    ee = etp.tile([P, 2 * QF], bf16, tag="e", name="e")
                    eo = etp.tile([P, 2 * QF], bf16, tag="e", name="e")
                    nc.scalar.activation(ee[:], spe[:], AF.Exp, scale=0.125)
                    nc.scalar.activation(eo[:], spo[:], AF.Exp, scale=0.125)
                    # attn@V for the previous k-chunk pair overlaps this
                    # pair's exp latency on the PE
                    if prev is not None:
                        pee, peo, pk = prev
                        for j in range(2):
                            kc = 2 * pk + j
                            js = slice(j * QF, (j + 1) * QF)
                            nc.tensor.matmul(
                                ope[:], vst[kc][:, 2 * pi, :], pee[:, js],
                                start=(kc == 0), stop=False)
                            nc.tensor.matmul(
                                opo[:], vst[kc][:, 2 * pi + 1, :], peo[:, js],
                                start=(kc == 0), stop=False)
                    prev = (ee, eo, kcp)
                pee, peo, pk = prev
                for j in range(2):
                    kc = 2 * pk + j
                    js = slice(j * QF, (j + 1) * QF)
                    nc.tensor.matmul(
                        ope[:], vst[kc][:, 2 * pi, :], pee[:, js],
                        start=False, stop=(kc == NT - 1))
                    nc.tensor.matmul(
                        opo[:], vst[kc][:, 2 * pi + 1, :], peo[:, js],
                        start=False, stop=(kc == NT - 1))

                # evict PSUM immediately (quick DVE copies free the "o"
                # slots), then flush the reciprocal chain right away --
                # its gpsimd/DVE latency hides under the next attention
                # block. The last head-pair's o-projection trails one
                # q-block so flush(qh-1) has a full block to complete.
                osb_e = npool.tile([HD + 1, QF], f32, tag="osb", name="osb",
                                   bufs=8)
                nc.vector.tensor_copy(osb_e[:], ope[:])
                osb_o = npool.tile([HD + 1, QF], f32, tag="osb", name="osb",
                                   bufs=8)
                nc.vector.tensor_copy(osb_o[:], opo[:])
                pend.append((pi, qs, osb_e, osb_o,
                             pi == NPI - 1 and qh == NQF - 1))
                flush_one()
                if pi == NPI - 1 and qh > 0:
                    oproj(qh - 1)
            cur, nxt = nxt, None

        oproj_last()
    nc.compile()
    return nc


def _rope_tables(pos):
    """pos [n] -> cos/sin tables [128, n] bf16 (sign-folded sin)."""
    inv = ROPE_BASE ** (-np.arange(0, HD, 2, dtype=np.float64) / HD)
    fr = np.outer(pos.astype(np.float64), inv)          # [n, 32]
    c, s = np.cos(fr), np.sin(fr)
    cos64 = np.concatenate([c, c], axis=1).T            # [64, n]
    sinA = np.concatenate([s, -s], axis=1).T            # [64, n]
    ck = np.concatenate([cos64, cos64], axis=0).astype(BF16)
    sk = np.concatenate([sinA, sinA], axis=0).astype(BF16)
    return ck, sk


def _prep_wqk(w, g):
    """[D, D] weight -> bf16 [P, NPI, ND, P]: W.T columns for head group
    g, prearranged so each per-pi stationary DMA is one contiguous 2KB
    descriptor per partition."""
    wt = np.asarray(w, dtype=np.float32).T[:, g * DO:(g + 1) * DO]
    arr = wt.reshape(ND, P, NPI, P).transpose(1, 2, 0, 3)
    return np.ascontiguousarray(arr).astype(BF16)


def _prep_wv(w, g):
    """[D, D] weight -> bf16 [P, ND, DO] (prearranged W.T columns)."""
    wt = np.asarray(w, dtype=np.float32).T[:, g * DO:(g + 1) * DO]
    arr = wt.reshape(ND, P, DO).transpose(1, 0, 2)
    return np.ascontiguousarray(arr).astype(BF16)


def _prep_wo(w, g):
    """[D, D] weight -> bf16 [P, NPI, D]: W.T rows for head group g."""
    wt = np.asarray(w, dtype=np.float32).T[g * DO:(g + 1) * DO, :]
    arr = wt.reshape(NPI, P, D).transpose(1, 0, 2)
    return np.ascontiguousarray(arr).astype(BF16)


def _prep_b(b, g):
    return np.ascontiguousarray(
        np.asarray(b, dtype=np.float32)[None, g * DO:(g + 1) * DO]).astype(BF16)


def kernel(hidden_states, position_ids, Wq, bq, Wk, bk, Wv, bv, Wo):
    from concourse import bass_utils

    with_bias = bool(
        np.any(np.asarray(bq)) or np.any(np.asarray(bk)) or np.any(np.asarray(bv)))
    key = ("nc", with_bias)
    if key not in _cache:
        _cache[key] = _build_nc(with_bias)
    nc = _cache[key]

    hs = np.asarray(hidden_states, dtype=np.float32)
    pos = np.asarray(position_ids)
    wq = [_prep_wqk(Wq, g) for g in range(2)]
    wk = [_prep_wqk(Wk, g) for g in range(2)]
    wv = [_prep_wv(Wv, g) for g in range(2)]
    wo = [_prep_wo(Wo, g) for g in range(2)]
    bqs = [_prep_b(bq, g) for g in range(2)]
    bks = [_prep_b(bk, g) for g in range(2)]
    bvs = [_prep_b(bv, g) for g in range(2)]

    xts, tabs = [], []
    for b in range(B):
        xT = np.empty((D + 1, SK), dtype=np.float32)
        xT[:D] = hs[b].T
        xT[D] = 1.0
        xts.append(np.ascontiguousarray(xT).astype(BF16))
        tabs.append(_rope_tables(np.asarray(pos[b])))

    in_maps = []
    for core in range(NCORES):
        b, g = core // 2, core % 2
        ck, sk = tabs[b]
        in_maps.append({
            "xT": xts[b], "wqT": wq[g], "wkT": wk[g], "wvT": wv[g],
            "woT": wo[g], "wqb": bqs[g], "wkb": bks[g], "wvb": bvs[g],
            "cosk": ck, "sink": sk,
        })

    res = bass_utils.run_bass_kernel_spmd(
        nc, in_maps, core_ids=list(range(NCORES)), trace=TRACE, **TRACE_KW)
    LAST["exec_time_ns"] = res.exec_time_ns
    LAST["mean_exec_time_ns"] = res.mean_exec_time_ns
    LAST["trace"] = res.instructions_and_trace
    LAST["profile_json"] = res.profile_json

    outp_full = np.empty((B, S, D), dtype=np.float32)
    for b in range(B):
        outp_full[b] = (
            np.asarray(res.results[2 * b]["out"], dtype=np.float32)
            + np.asarray(res.results[2 * b + 1]["out"], dtype=np.float32))
    return outp_full
